# revision 21
# baseline (speedup 1.0000x reference)
"""Trainium2 Bass kernel for DiffeomorphicLearner (gnn_message_passing).

Math (per step t, T=8 steps):
    sq_i  = ||z_i||^2
    K_ij  = exp((2 z_i.z_j - sq_i - sq_j) / (2 rho^2))
    v     = Z @ Aaff_t.T + b_t + K @ A_t
    Z    <- Z + DT * v

Distribution: row-parallel over N=4096 across 8 cores (512 rows each).
Each step every core needs the full updated Z (fp8, transposed) plus the
per-row sq.  Instead of a collective AllGather (13-16us on the critical
path + unpack DMAs), each core pushes its 1032B/partition payload
(z fp8 ch0|ch1 + sq bf16 bits) straight into the peers' SBUF receive
buffer with 7 remote_dma_broadcast descriptors, and consumers gate on a
monotonic semaphore that counts remote arrivals (14 = 7 senders x 2).

Slot map: receive buffer slot s holds rank (me ^ rho(s)) where
rho(s)=s for s<4 and s^2 for s>=4 -- the D2D first hop lands on the
diagonal SEngine (SE0<->SE3, SE1<->SE2), XOR-2 off the naive relative
map (verified on HW).  The j-reduction is permutation invariant, so the
host feeds each core A rows permuted into its own slot order.

Precision: all matmuls fp8 DoubleRow in / fp32 PSUM accumulate; exp
argument and state updates fp32.  exp(-c*sq_j) is folded into A
(per-partition DVE rescale); exp(-c*sq_i) post-multiplies the K@A sum.

Scheduling: the j-block loop is software-pipelined with a skew -- the
S = Z_j.Z_i matmuls and the exp run PIPE_LAG block-pairs ahead of the
K@A consumer matmuls.  Slot-0 (local) pairs run first so the dense
phase starts before remote slices land.
"""

import numpy as np
import ml_dtypes

import concourse.bass as bass
import concourse.tile as tile
from concourse import bacc, mybir
from concourse import bass_utils

BF16NP = ml_dtypes.bfloat16
F8NP = ml_dtypes.float8_e4m3

N_CORES = 8
N, D, T = 4096, 256, 8
RHO = 16.0
DT = 1.0 / T
CEXP = 1.0 / (2.0 * RHO * RHO)  # 1/512
SA = 64.0                      # fp8 scale for A (values are subnormal otherwise)
SAF = 64.0                     # fp8 scale for A_aff / b_aff

NLOC = N // N_CORES            # 512 rows per core
NJB = N // 128                 # 32 j-blocks of 128
NJB_LOC = NLOC // 128          # 4 local i-blocks
SLOT = 2 * NLOC + 8            # 1032 fp8 bytes: z ch0 | z ch1 | sq bf16 bits
PIPE_LAG = 2                   # block-PAIR skew between S/exp and K@A matmuls
FILLER_MMS = 1                 # E-broadcast repetitions in the gather window
PRE_FILLERS = 1                # E-broadcast repetitions before the update

F32 = mybir.dt.float32
BF16 = mybir.dt.bfloat16
F8 = mybir.dt.float8e4
DR = mybir.MatmulPerfMode.DoubleRow

# slot s <- rank (me ^ RHO_MAP[s]); the D2D hop lands on the diagonal
# SEngine (HW-verified), XOR-2 off the naive map.  KERNEL_RHO_ID=1
# switches the host-side permutation to the naive map (what the
# multi-core simulator models).
import os
if os.environ.get("KERNEL_RHO_ID"):
    RHO_MAP = list(range(N_CORES))
else:
    RHO_MAP = [s if s < 4 else s ^ 2 for s in range(N_CORES)]

_CACHED = {}


def _build():
    """Build the 8-core SPMD Bass program (same program on every core)."""
    nc = bacc.Bacc("TRN2", target_bir_lowering=False, debug=False,
                   num_devices=N_CORES, monotonic_sem_count=2 * (T - 1))

    # per-gather-step semaphores with CONSTANT wait targets: arrivals
    # (+2 per sender, 7 senders -> 14) and local send completion
    # (+16 per broadcast, 7 broadcasts -> 112).  Constant targets live in
    # preamble-initialized registers so the attached waits are
    # register-valued (the scheduling pass treats those optimistically)
    # yet nothing in the loop ever mutates them.
    sem_data = [nc.monotonic_semaphore(k).sem() for k in range(T - 1)]
    sem_sent = [nc.monotonic_semaphore(T - 1 + k).sem() for k in range(T - 1)]
    vreg14 = nc.vector.alloc_register("tgt14")
    nc.vector.reg_mov(vreg14, 14)
    vreg112 = nc.vector.alloc_register("tgt112")
    nc.vector.reg_mov(vreg112, 112)

    # ---- DRAM I/O -------------------------------------------------------
    zt_local0 = nc.dram_tensor("zt_local0", [D, NLOC], F32, kind="ExternalInput")
    ztr_init = nc.dram_tensor("ztr_init", [128, N_CORES * SLOT], F8,
                              kind="ExternalInput")
    e_row0 = nc.dram_tensor("e_row0", [1, NLOC], BF16, kind="ExternalInput")
    a_b = nc.dram_tensor("a_b", [T, N, D], F8, kind="ExternalInput")
    aaff_b = nc.dram_tensor("aaff_b", [T, D, D], F8, kind="ExternalInput")
    b_b = nc.dram_tensor("b_b", [T, 1, D], BF16, kind="ExternalInput")
    ones_col = nc.dram_tensor("ones_col", [128, 1], BF16, kind="ExternalInput")
    ones_row = nc.dram_tensor("ones_row", [1, NLOC], BF16, kind="ExternalInput")
    log_inv_sa = nc.dram_tensor("log_inv_sa", [1, 1], F32, kind="ExternalInput")
    out_zt = nc.dram_tensor("out_zt", [D, NLOC], F32, kind="ExternalOutput")

    EXP = mybir.ActivationFunctionType.Exp
    SQUARE = mybir.ActivationFunctionType.Square

    with tile.TileContext(nc) as tc:
        with tc.tile_pool(name="persist", bufs=1) as persist, \
             tc.tile_pool(name="state", bufs=2) as state, \
             tc.tile_pool(name="astream", bufs=2) as astream, \
             tc.tile_pool(name="kpool", bufs=6) as kpool, \
             tc.tile_pool(name="work", bufs=2) as work, \
             tc.tile_pool(name="psum", bufs=1, space="PSUM") as psum:

            # ---- constants / persistent buffers -------------------------
            onec = persist.tile([128, 1], BF16, name="onec")
            nc.sync.dma_start(onec[:], ones_col[:])
            oner = persist.tile([1, NLOC], BF16, name="oner")
            nc.sync.dma_start(oner[:], ones_row[:])
            lsa = persist.tile([1, 1], F32, name="lsa")
            nc.sync.dma_start(lsa[:], log_inv_sa[:])

            # double-buffered gathered state: slot s = [z ch0 | z ch1 | sq]
            ztr = [persist.tile([128, N_CORES * SLOT], F8, name=f"ztr{p}")
                   for p in (0, 1)]
            nc.sync.dma_start(ztr[0][:], ztr_init[:])

            # local state: fp32 master
            zt = [state.tile([128, NLOC], F32, name=f"zt{ch}", tag=f"zt{ch}")
                  for ch in (0, 1)]
            for ch in (0, 1):
                nc.sync.dma_start(zt[ch][:], zt_local0[ch * 128:(ch + 1) * 128, :])

            e_row = state.tile([1, NLOC], BF16, name="e_row", tag="e_row")
            nc.sync.dma_start(e_row[:], e_row0[:])

            e_col = state.tile([128, NJB], F32, name="e_col0", tag="e_col")
            sq_gather = state.tile([128, NJB], BF16, name="sqg0", tag="sqg")
            nc.vector.tensor_copy(
                sq_gather[:],
                ztr[0][:].bitcast(BF16).rearrange(
                    "p (s w) -> p s w", s=N_CORES)[:, :, 512:516])
            nc.scalar.activation(e_col[:], sq_gather[:], EXP, scale=-CEXP)

            for t in range(T):
                last = (t == T - 1)
                cur = ztr[t % 2]
                nxt = ztr[1 - t % 2]

                # ---- A_t: one 2 MB DMA into [128, 32*256] ---------------
                a_sb = astream.tile([128, NJB * D], F8, name=f"a_{t}", tag="a")
                nc.sync.dma_start(
                    a_sb[:].rearrange("p (j d) -> p j d", j=NJB),
                    a_b.ap()[t].rearrange("(j p) d -> p j d", p=128))

                # Aaff_t: one DMA into [128, 2*256]; b_t row
                aaff_sb = astream.tile([128, 2 * D], F8, name=f"aaff_{t}",
                                       tag="aaff")
                nc.sync.dma_start(
                    aaff_sb[:].rearrange("p (c d) -> p c d", c=2),
                    aaff_b.ap()[t].rearrange("(c p) d -> p c d", p=128))
                brow_t = astream.tile([1, D], BF16, name=f"brow_{t}", tag="brow")
                nc.sync.dma_start(brow_t[:], b_b.ap()[t, :, :])

                # ---- E broadcast: E[p, i] = exp(-c*sq_i) ----------------
                if t == 0:
                    e_ps = psum.tile([128, NLOC], F32, name="e_ps_0",
                                     tag="aux", bufs=2)
                    nc.tensor.matmul(e_ps[:], oner[:, 0:128], e_row[:],
                                     start=True, stop=True)
                    e_sb = work.tile([128, NLOC], F32, name="e_sb_0",
                                     tag="e_sb", bufs=2)
                    nc.vector.tensor_copy(e_sb[:], e_ps[:])

                # local z (slot 0 of cur) is the rhs for S and affine
                zloc = cur[:, 0:2 * NLOC]
                zloc3 = zloc.rearrange("k (r i) -> k r i", r=2)

                # ---- affine part: va[dh] = Aaff_t @ z_loc + b_t ---------
                va_sb = []
                aaff3 = aaff_sb[:].rearrange("k (r d) -> k r d", r=2)
                for dh in (0, 1):
                    va = psum.tile([128, NLOC], F32, name=f"va_{t}_{dh}",
                                   tag="aux", bufs=2)
                    nc.tensor.matmul(va[:],
                                     aaff3[:, :, dh * 128:(dh + 1) * 128],
                                     zloc3[:], start=True, stop=False,
                                     perf_mode=DR)
                    nc.tensor.matmul(va[:],
                                     brow_t[:, dh * 128:(dh + 1) * 128],
                                     oner[:], start=False, stop=True)
                    # va holds SAF*(Aaff@z + b): descale, then add z master
                    vs0 = work.tile([128, NLOC], F32, name=f"vs0_{t}_{dh}",
                                    tag=f"vs0{dh}", bufs=2)
                    nc.vector.tensor_scalar_mul(vs0[:], va[:], 1.0 / SAF)
                    vsb = work.tile([128, NLOC], F32, name=f"vasb_{t}_{dh}",
                                    tag=f"vasb{dh}", bufs=2)
                    nc.vector.tensor_add(vsb[:], vs0[:], zt[dh][:])
                    va_sb.append(vsb)

                # Remote data must be consumed through a Tile-visible,
                # arrival-gated producer: LDWEIGHTS only inherits deps from
                # the weights AP, and the only local "writer" of the remote
                # slots is the descriptor-gen prep (which runs long before
                # the data lands).  One bf16 4x-mode copy of the WHOLE
                # receive buffer (slot 0 included, so it data-depends on
                # the full send chain of step t-1 -- dropping slot 0
                # recreates a global send-blocks-arrival deadlock) carries
                # the arrival wait; everything remote reads the copy.
                if t > 0:
                    ztc = work.tile([128, N_CORES * SLOT], F8,
                                    name=f"ztc_{t}", tag="ztc", bufs=2)
                    cp = nc.vector.tensor_copy(ztc[:].bitcast(BF16),
                                               cur[:].bitcast(BF16))
                    cp.wait_op(sem_data[t - 1], vreg14, "sem-ge")
                    rsrc = ztc
                    # e_col for this step from the gathered sq tails
                    sq_g = state.tile([128, NJB], BF16, name=f"sqg_{t}",
                                      tag="sqg")
                    nc.vector.tensor_copy(
                        sq_g[:],
                        rsrc[:].bitcast(BF16).rearrange(
                            "p (s w) -> p s w", s=N_CORES)[:, :, 512:516])
                    e_col = state.tile([128, NJB], F32, name=f"ec_{t}",
                                       tag="e_col")
                    nc.scalar.activation(e_col[:], sq_g[:], EXP, scale=-CEXP)
                else:
                    rsrc = cur

                # ---- A' = A * exp(-c*sq_j): per-partition rescale -------
                a_sc = astream.tile([128, NJB * D], F8, name=f"asc_{t}",
                                    tag="asc")
                for jb in range(NJB):
                    nc.vector.tensor_scalar_mul(
                        a_sc[:, jb * D:(jb + 1) * D],
                        a_sb[:, jb * D:(jb + 1) * D],
                        e_col[:, jb:jb + 1])

                # ---- main loop over block PAIRS (DoubleRow, fp8) --------
                NPAIR = NJB // 2
                vr = [psum.tile([128, NLOC], F32, name=f"vr_{t}_{dh}",
                                tag=f"vr{dh}", bufs=1) for dh in (0, 1)]
                k_pairs = [None] * NPAIR
                for qq in range(NPAIR + PIPE_LAG):
                    if qq < NPAIR:
                        local = (qq < 2)
                        k_p = kpool.tile([128, 2 * NLOC], F8,
                                         name=f"k_{t}_{qq}", tag="k")
                        s_ps = psum.tile([128, 2 * NLOC], F32,
                                         name=f"s_{t}_{qq}",
                                         tag="s", bufs=2)
                        for h in (0, 1):
                            jb = 2 * qq + h
                            s_idx, lb = jb // 4, jb % 4
                            wbase = cur if local else rsrc
                            wsl = wbase[:, s_idx * SLOT:
                                        s_idx * SLOT + 2 * NLOC].rearrange(
                                "k (r j) -> k r j", r=2)[
                                :, :, lb * 128:(lb + 1) * 128]
                            nc.tensor.matmul(
                                s_ps[:, h * NLOC:(h + 1) * NLOC],
                                wsl,
                                zloc3[:],
                                start=True, stop=True,
                                perf_mode=DR)
                        # G = exp(2cS); exp(-c sq_j) is pre-folded into A,
                        # exp(-c sq_i) post-multiplies vr
                        nc.scalar.activation(k_p[:], s_ps[:],
                                             EXP, scale=2.0 * CEXP)
                        k_pairs[qq] = k_p
                    if qq >= PIPE_LAG:
                        q = qq - PIPE_LAG
                        k_p = k_pairs[q]
                        a3 = a_sc[:, 2 * q * D:2 * (q + 1) * D].rearrange(
                            "k (r d) -> k r d", r=2)
                        k3 = k_p[:].rearrange("k (r i) -> k r i", r=2)
                        for dh in (0, 1):
                            nc.tensor.matmul(
                                vr[dh][:],
                                a3[:, :, dh * 128:(dh + 1) * 128],
                                k3[:],
                                start=(q == 0),
                                stop=(q == NPAIR - 1),
                                perf_mode=DR)

                # keep TensorE busy across the DVE update window (the
                # clock-gate re-throttles on any PE idle)
                if not last:
                    e_ps_next = psum.tile([128, NLOC], F32,
                                          name=f"e_ps_{t + 1}",
                                          tag="aux", bufs=2)
                    for _f in range(PRE_FILLERS):
                        nc.tensor.matmul(e_ps_next[:], oner[:, 0:128],
                                         e_row[:], start=True, stop=True)

                # ---- update: z <- z + va + vr * E -----------------------
                # fp32 master + fp8 copy straight into nxt slot 0
                zt_new = [state.tile([128, NLOC], F32, name=f"ztn_{t}_{ch}",
                                     tag=f"zt{ch}") for ch in (0, 1)]
                for dh in (0, 1):
                    t1 = work.tile([128, NLOC], F32, name=f"t1_{t}_{dh}",
                                   tag="t1", bufs=2)
                    nc.vector.tensor_mul(t1[:], vr[dh][:], e_sb[:])
                    nc.vector.tensor_add(zt_new[dh][:], t1[:], va_sb[dh][:])
                zt = zt_new

                if last:
                    for ch in (0, 1):
                        nc.sync.dma_start(
                            out_zt[ch * 128:(ch + 1) * 128, :], zt[ch][:])
                    break

                # fp8 state into nxt slot 0 (also the send payload).
                # WAR guard: sends of step t-2 must have drained (7x16 each)
                for ch in (0, 1):
                    cpz = nc.vector.tensor_copy(
                        nxt[:, ch * NLOC:(ch + 1) * NLOC], zt[ch][:])
                    if ch == 0 and t >= 2:
                        cpz.wait_op(sem_sent[t - 2], vreg112, "sem-ge")

                # ---- sq of new z: column layout bf16 into slot 0 tail ---
                z2 = [work.tile([128, NLOC], BF16, name=f"z2_{t}_{ch}",
                                tag=f"z2{ch}", bufs=2) for ch in (0, 1)]
                for ch in (0, 1):
                    nc.scalar.activation(z2[ch][:], zt[ch][:], SQUARE)
                sqc_ps = psum.tile([128, NJB_LOC], F32, name=f"sqc_{t}",
                                   tag="aux", bufs=2)
                for ib in range(NJB_LOC):
                    for ch in (0, 1):
                        nc.tensor.matmul(sqc_ps[:, ib:ib + 1],
                                         z2[ch][:, ib * 128:(ib + 1) * 128],
                                         onec[:],
                                         start=(ch == 0), stop=(ch == 1))
                nc.vector.tensor_copy(
                    nxt[:].bitcast(BF16)[:, 512:516], sqc_ps[:])

                # ---- send slot 0 to the 7 peers -------------------------
                for s in range(1, N_CORES):
                    rdests = [None] * N_CORES
                    rdests[s] = (0, s)
                    nc.gpsimd.remote_dma_broadcast(
                        nxt[:, s * SLOT:s * SLOT + SLOT],
                        nxt[:, 0:SLOT],
                        sem_data[t], sem_sent[t],
                        rdests=rdests,
                    )
                nc.gpsimd.trigger_dma(count=None)

                # ---- post-trigger: e_row/e_sb for next step -------------
                sqr_ps = psum.tile([1, NLOC], F32, name=f"sqr_{t}",
                                   tag="aux", bufs=2)
                for ch in (0, 1):
                    nc.tensor.matmul(sqr_ps[:], onec[:], z2[ch][:],
                                     start=(ch == 0), stop=(ch == 1))
                e_row_new = state.tile([1, NLOC], BF16, name=f"er_{t}",
                                       tag="e_row")
                nc.scalar.activation(e_row_new[:], sqr_ps[:], EXP, scale=-CEXP,
                                     bias=lsa[:])
                e_row = e_row_new

                # next step's E broadcast + HAM keep-warm fillers
                for _f in range(FILLER_MMS):
                    nc.tensor.matmul(e_ps_next[:], oner[:, 0:128], e_row[:],
                                     start=True, stop=True)
                e_sb = work.tile([128, NLOC], F32, name=f"e_sb_{t + 1}",
                                 tag="e_sb", bufs=2)
                nc.vector.tensor_copy(e_sb[:], e_ps_next[:])

    nc.compile()
    return nc


def _prepare_in_maps(X, A, A_aff, b_aff):
    XT = np.ascontiguousarray(X.T.astype(np.float32))          # [D, N]
    XT8 = XT.astype(F8NP)
    sq0 = (X.astype(np.float32) ** 2).sum(axis=1)              # [N]
    sq0_col = sq0.reshape(N_CORES, NJB_LOC, 128)               # [rank, ib, p]
    a_sc = DT * SA * A.astype(np.float32)                      # [T, N, D]
    aaff_b = np.ascontiguousarray(
        (DT * SAF * A_aff.astype(np.float32)).transpose(0, 2, 1)).astype(F8NP)
    b_b = (DT * SAF * b_aff.astype(np.float32)).reshape(T, 1, D).astype(BF16NP)
    ones_col = np.ones((128, 1), dtype=BF16NP)
    ones_row = np.ones((1, NLOC), dtype=BF16NP)

    in_maps = []
    for c in range(N_CORES):
        cols = slice(c * NLOC, (c + 1) * NLOC)
        # slot layout per core: slot s holds rank c ^ RHO_MAP[s]
        ztr_init = np.zeros((128, N_CORES * SLOT), dtype=F8NP)
        perm = np.empty(N, dtype=np.int64)
        for s in range(N_CORES):
            q = c ^ RHO_MAP[s]
            qc = slice(q * NLOC, (q + 1) * NLOC)
            for ch in (0, 1):
                ztr_init[:, s * SLOT + ch * NLOC:
                         s * SLOT + (ch + 1) * NLOC] = \
                    XT8[ch * 128:(ch + 1) * 128, qc]
            sq_bits = sq0_col[q].astype(BF16NP).T.copy()       # [128, 4]
            ztr_init[:, s * SLOT + 2 * NLOC:
                     s * SLOT + 2 * NLOC + 8] = \
                sq_bits.view(np.uint8).view(F8NP)
            perm[s * NLOC:(s + 1) * NLOC] = np.arange(q * NLOC, (q + 1) * NLOC)
        a_b = np.ascontiguousarray(a_sc[:, perm, :]).astype(F8NP)
        in_maps.append({
            "zt_local0": np.ascontiguousarray(XT[:, cols]),
            "ztr_init": ztr_init,
            "e_row0": (np.exp(-CEXP * sq0[cols]) / SA)[None, :].astype(BF16NP),
            "a_b": a_b,
            "aaff_b": aaff_b,
            "b_b": b_b,
            "ones_col": ones_col,
            "ones_row": ones_row,
            "log_inv_sa": np.array([[np.log(1.0 / SA)]], dtype=np.float32),
        })
    return in_maps


def _get_nc():
    if "nc" not in _CACHED:
        _CACHED["nc"] = _build()
    return _CACHED["nc"]


def kernel(X, A, A_aff, b_aff):
    X = np.asarray(X)
    A = np.asarray(A)
    A_aff = np.asarray(A_aff)
    b_aff = np.asarray(b_aff)
    nc = _get_nc()
    in_maps = _prepare_in_maps(X, A, A_aff, b_aff)
    res = bass_utils.run_bass_kernel_spmd(
        nc, in_maps, core_ids=list(range(N_CORES)))
    out = np.empty((N, D), dtype=np.float32)
    for c in range(N_CORES):
        out[c * NLOC:(c + 1) * NLOC, :] = res.results[c]["out_zt"].T
    return out


# revision 25
# speedup vs baseline: 22.3184x; 22.3184x over previous
"""Trainium2 Bass kernel for DiffeomorphicLearner (gnn_message_passing).

Math (per step t, T=8 steps):
    sq_i  = ||z_i||^2
    K_ij  = exp((2 z_i.z_j - sq_i - sq_j) / (2 rho^2))
    v     = Z @ Aaff_t.T + b_t + K @ A_t
    Z    <- Z + DT * v

Distribution: row-parallel over N=4096 across 8 cores (512 rows each).
Each core keeps its Z rows (fp32 master, stored TRANSPOSED as [D, n_loc])
and computes K^T slices [j, i_loc] against an all-gathered bf16 copy of
the full Z^T plus column-layout sq. One bf16 AllGather of [Z^T; sq]
(257 x 512 per rank) per step.

Precision: all matmuls bf16 in / fp32 PSUM accumulate; exp argument and
state updates fp32. sq_j enters as a per-partition ACT bias; the
exp(-c*sq_i) factor is factored out per-column and applied after the
K@A contraction (it is constant along j).

Scheduling: the j-block loop is software-pipelined with a skew — the
S = Z_j.Z_i matmuls and the exp run PIPE_LAG blocks ahead of the K@A
consumer matmuls, so the TensorE stream never stalls on ScalarE.
Bulk DRAM traffic (A_t stream, AllGather unpack) is issued as single
strided DMA instructions to avoid Sync-engine issue serialization.
"""

import numpy as np
import ml_dtypes

import concourse.bass as bass
import concourse.tile as tile
from concourse import bacc, mybir
from concourse import bass_utils

BF16NP = ml_dtypes.bfloat16
F8NP = ml_dtypes.float8_e4m3

N_CORES = 8
N, D, T = 4096, 256, 8
RHO = 16.0
DT = 1.0 / T
CEXP = 1.0 / (2.0 * RHO * RHO)  # 1/512
SA = 64.0                      # fp8 scale for A (values are subnormal otherwise)
SAF = 64.0                     # fp8 scale for A_aff / b_aff

NLOC = N // N_CORES            # 512 rows per core
NJB = N // 128                 # 32 j-blocks of 128
NJB_LOC = NLOC // 128          # 4 local i-blocks
PAY_R = D + 2                  # payload rows: 256 fp8 Z rows + 2 rows of bf16 sq bits
PIPE_LAG = 2                   # block-PAIR skew between S/exp and K@A matmuls
FILLER_MMS = 1                 # E-broadcast repetitions in the AG window
PRE_FILLERS = 1                # E-broadcast repetitions before the update

F32 = mybir.dt.float32
BF16 = mybir.dt.bfloat16
F8 = mybir.dt.float8e4
DR = mybir.MatmulPerfMode.DoubleRow

_CACHED = {}


def _build():
    """Build the 8-core SPMD Bass program (same program on every core)."""
    nc = bacc.Bacc("TRN2", target_bir_lowering=False, debug=False,
                   num_devices=N_CORES)

    # ---- DRAM I/O -------------------------------------------------------
    zt_local0 = nc.dram_tensor("zt_local0", [D, NLOC], F32, kind="ExternalInput")
    zb_local0 = nc.dram_tensor("zb_local0", [D, NLOC], F8, kind="ExternalInput")
    ztb_full0 = nc.dram_tensor("ztb_full0", [D, N], F8, kind="ExternalInput")
    bias_col0 = nc.dram_tensor("bias_col0", [128, NJB], F32, kind="ExternalInput")
    e_row0 = nc.dram_tensor("e_row0", [1, NLOC], BF16, kind="ExternalInput")
    a_b = nc.dram_tensor("a_b", [T, N, D], F8, kind="ExternalInput")
    aaff_b = nc.dram_tensor("aaff_b", [T, D, D], F8, kind="ExternalInput")
    b_b = nc.dram_tensor("b_b", [T, 1, D], BF16, kind="ExternalInput")
    ones_col = nc.dram_tensor("ones_col", [128, 1], BF16, kind="ExternalInput")
    ones_row = nc.dram_tensor("ones_row", [1, NLOC], BF16, kind="ExternalInput")
    log_inv_sa = nc.dram_tensor("log_inv_sa", [1, 1], F32, kind="ExternalInput")
    out_zt = nc.dram_tensor("out_zt", [D, NLOC], F32, kind="ExternalOutput")

    EXP = mybir.ActivationFunctionType.Exp
    SQUARE = mybir.ActivationFunctionType.Square
    COPY = mybir.ActivationFunctionType.Copy

    with tile.TileContext(nc) as tc:
        with tc.tile_pool(name="persist", bufs=1) as persist, \
             tc.tile_pool(name="state", bufs=2) as state, \
             tc.tile_pool(name="astream", bufs=2) as astream, \
             tc.tile_pool(name="kpool", bufs=6) as kpool, \
             tc.tile_pool(name="work", bufs=2) as work, \
             tc.tile_pool(name="psum", bufs=1, space="PSUM") as psum, \
             tc.tile_pool(name="dram", bufs=2, space="DRAM") as dram:

            # ---- constants / persistent buffers -------------------------
            onec = persist.tile([128, 1], BF16, name="onec")
            nc.sync.dma_start(onec[:], ones_col[:])
            oner = persist.tile([1, NLOC], BF16, name="oner")
            nc.sync.dma_start(oner[:], ones_row[:])
            lsa = persist.tile([1, 1], F32, name="lsa")
            nc.sync.dma_start(lsa[:], log_inv_sa[:])

            # full gathered Z^T (bf16), rewritten each step by unpack DMAs
            ztf = persist.tile([128, 2 * N], F8, name="ztf")
            for ch in (0, 1):
                nc.sync.dma_start(ztf[:, ch * N:(ch + 1) * N],
                                  ztb_full0[ch * 128:(ch + 1) * 128, :])

            # gathered sq in column layout -> prescaled ACT bias (-c * sq)
            bias_col = persist.tile([128, NJB], F32, name="bias_col")
            nc.sync.dma_start(bias_col[:], bias_col0[:])
            sqc_all = persist.tile([128, NJB], BF16, name="sqc_all")

            # local state: fp32 master + bf16 working copy
            zt = [state.tile([128, NLOC], F32, name=f"zt{ch}", tag=f"zt{ch}")
                  for ch in (0, 1)]
            for ch in (0, 1):
                nc.sync.dma_start(zt[ch][:], zt_local0[ch * 128:(ch + 1) * 128, :])
            zb = state.tile([128, 2 * NLOC], F8, name="zb", tag="zb")
            for ch in (0, 1):
                nc.sync.dma_start(zb[:, ch * NLOC:(ch + 1) * NLOC],
                                  zb_local0[ch * 128:(ch + 1) * 128, :])

            e_row = state.tile([1, NLOC], BF16, name="e_row", tag="e_row")
            nc.sync.dma_start(e_row[:], e_row0[:])

            # e_col = exp(-c*sq_j) in column layout (bias_col is -c*sq)
            e_col = persist.tile([128, NJB], F32, name="e_col")
            nc.scalar.activation(e_col[:], bias_col[:], EXP)

            # warm-up AllGather: absorbs the ~90us first-collective setup
            # concurrently with step-0 compute (nothing consumes its output)
            cc_warm_in = dram.tile([1, NLOC], BF16, name="cc_warm_in",
                                   bufs=1)
            cc_warm_out = dram.tile([N_CORES, NLOC], BF16, name="cc_warm_out",
                                    bufs=1, addr_space="Shared")
            nc.sync.dma_start(cc_warm_in[:], oner[:])
            nc.gpsimd.collective_compute(
                "AllGather", mybir.AluOpType.bypass,
                replica_groups=[list(range(N_CORES))],
                ins=[cc_warm_in[:].opt()], outs=[cc_warm_out[:].opt()],
            )

            for t in range(T):
                last = (t == T - 1)

                # ---- A_t: one 2 MB DMA into [128, 32*256] ---------------
                a_sb = astream.tile([128, NJB * D], F8, name=f"a_{t}", tag="a")
                nc.sync.dma_start(
                    a_sb[:].rearrange("p (j d) -> p j d", j=NJB),
                    a_b.ap()[t].rearrange("(j p) d -> p j d", p=128))

                # Aaff_t: one DMA into [128, 2*256]; b_t row
                aaff_sb = astream.tile([128, 2 * D], F8, name=f"aaff_{t}",
                                       tag="aaff")
                nc.sync.dma_start(
                    aaff_sb[:].rearrange("p (c d) -> p c d", c=2),
                    aaff_b.ap()[t].rearrange("(c p) d -> p c d", p=128))
                brow_t = astream.tile([1, D], BF16, name=f"brow_{t}", tag="brow")
                nc.sync.dma_start(brow_t[:], b_b.ap()[t, :, :])

                # ---- E broadcast: E[p, i] = exp(-c*sq_i) ----------------
                # (steps >= 1 build e_sb in the previous step's tail, where
                # the repeated idempotent matmul doubles as HAM keep-warm
                # filler during the AllGather wait)
                if t == 0:
                    e_ps = psum.tile([128, NLOC], F32, name="e_ps_0",
                                     tag="aux", bufs=2)
                    nc.tensor.matmul(e_ps[:], oner[:, 0:128], e_row[:],
                                     start=True, stop=True)
                    e_sb = work.tile([128, NLOC], F32, name="e_sb_0",
                                     tag="e_sb", bufs=2)
                    nc.vector.tensor_copy(e_sb[:], e_ps[:])

                # ---- affine part: va[dh] = Aaff_t @ z_loc + b_t ---------
                va_sb = []
                aaff3 = aaff_sb[:].rearrange("k (r d) -> k r d", r=2)
                zb3a = zb[:].rearrange("k (r i) -> k r i", r=2)
                for dh in (0, 1):
                    va = psum.tile([128, NLOC], F32, name=f"va_{t}_{dh}",
                                   tag="aux", bufs=2)
                    nc.tensor.matmul(va[:],
                                     aaff3[:, :, dh * 128:(dh + 1) * 128],
                                     zb3a[:], start=True, stop=False,
                                     perf_mode=DR)
                    nc.tensor.matmul(va[:],
                                     brow_t[:, dh * 128:(dh + 1) * 128],
                                     oner[:], start=False, stop=True)
                    # va holds SAF*(Aaff@z + b): descale, then add z master
                    vs0 = work.tile([128, NLOC], F32, name=f"vs0_{t}_{dh}",
                                    tag=f"vs0{dh}", bufs=2)
                    nc.vector.tensor_scalar_mul(vs0[:], va[:], 1.0 / SAF)
                    vsb = work.tile([128, NLOC], F32, name=f"vasb_{t}_{dh}",
                                    tag=f"vasb{dh}", bufs=2)
                    nc.vector.tensor_add(vsb[:], vs0[:], zt[dh][:])
                    va_sb.append(vsb)

                # ---- A' = A * exp(-c*sq_j): per-partition rescale -------
                a_sc = astream.tile([128, NJB * D], F8, name=f"asc_{t}",
                                    tag="asc")
                for jb in range(NJB):
                    nc.vector.tensor_scalar_mul(
                        a_sc[:, jb * D:(jb + 1) * D],
                        a_sb[:, jb * D:(jb + 1) * D],
                        e_col[:, jb:jb + 1])

                # ---- main loop over block PAIRS (DoubleRow, fp8) --------
                NPAIR = NJB // 2
                ztf3 = ztf[:].rearrange("k (r j) -> k r j", r=2)
                zb3 = zb[:].rearrange("k (r i) -> k r i", r=2)
                vr = [psum.tile([128, NLOC], F32, name=f"vr_{t}_{dh}",
                                tag=f"vr{dh}", bufs=1) for dh in (0, 1)]
                k_pairs = [None] * NPAIR
                for qq in range(NPAIR + PIPE_LAG):
                    if qq < NPAIR:
                        k_p = kpool.tile([128, 2 * NLOC], F8,
                                         name=f"k_{t}_{qq}", tag="k")
                        s_ps = psum.tile([128, 2 * NLOC], F32,
                                         name=f"s_{t}_{qq}",
                                         tag="s", bufs=2)
                        for h in (0, 1):
                            jb = 2 * qq + h
                            nc.tensor.matmul(
                                s_ps[:, h * NLOC:(h + 1) * NLOC],
                                ztf3[:, :, jb * 128:(jb + 1) * 128],
                                zb3[:], start=True, stop=True,
                                perf_mode=DR)
                        # G = exp(2cS); exp(-c sq_j) is pre-folded into A,
                        # exp(-c sq_i) post-multiplies vr
                        nc.scalar.activation(k_p[:], s_ps[:],
                                             EXP, scale=2.0 * CEXP)
                        k_pairs[qq] = k_p
                    if qq >= PIPE_LAG:
                        q = qq - PIPE_LAG
                        k_p = k_pairs[q]
                        a3 = a_sc[:, 2 * q * D:2 * (q + 1) * D].rearrange(
                            "k (r d) -> k r d", r=2)
                        k3 = k_p[:].rearrange("k (r i) -> k r i", r=2)
                        for dh in (0, 1):
                            nc.tensor.matmul(
                                vr[dh][:],
                                a3[:, :, dh * 128:(dh + 1) * 128],
                                k3[:],
                                start=(q == 0),
                                stop=(q == NPAIR - 1 and dh == 1),
                                perf_mode=DR)

                # keep TensorE busy across the DVE update window (the
                # clock-gate re-throttles on any PE idle) — idempotent,
                # overwritten by the real fillers in the tail
                if not last:
                    e_ps_next = psum.tile([128, NLOC], F32,
                                          name=f"e_ps_{t + 1}",
                                          tag="aux", bufs=2)
                    for _f in range(PRE_FILLERS):
                        nc.tensor.matmul(e_ps_next[:], oner[:, 0:128],
                                         e_row[:], start=True, stop=True)

                # ---- update: z <- z + va + vr * E -----------------------
                zt_new = [state.tile([128, NLOC], F32, name=f"ztn_{t}_{ch}",
                                     tag=f"zt{ch}") for ch in (0, 1)]
                for dh in (0, 1):
                    t1 = work.tile([128, NLOC], F32, name=f"t1_{t}_{dh}",
                                   tag="t1", bufs=2)
                    nc.vector.tensor_mul(t1[:], vr[dh][:], e_sb[:])
                    nc.vector.tensor_add(zt_new[dh][:], t1[:], va_sb[dh][:])
                zt = zt_new

                if last:
                    for ch in (0, 1):
                        nc.sync.dma_start(
                            out_zt[ch * 128:(ch + 1) * 128, :], zt[ch][:])
                    break

                # ---- post-update tail: bf16 copy, sq, payload, AG -------
                # critical path to the AG doorbell: zb cast -> pack DMA per
                # channel as soon as it lands, sq column path, sq pack.
                # Everything else (e_row, unpack prep) goes after the
                # trigger so the collective starts as early as possible.
                cc_in = dram.tile([PAY_R, NLOC], F8, name=f"cci_{t}",
                                  tag="cci")
                cc_out = dram.tile([N_CORES * PAY_R, NLOC], F8,
                                   name=f"cco_{t}", tag="cco",
                                   addr_space="Shared")
                zb_new = state.tile([128, 2 * NLOC], F8, name=f"zbn_{t}",
                                    tag="zb")
                z2 = [work.tile([128, NLOC], BF16, name=f"z2_{t}_{ch}",
                                tag=f"z2{ch}", bufs=2) for ch in (0, 1)]
                for ch in (0, 1):
                    nc.vector.tensor_copy(
                        zb_new[:, ch * NLOC:(ch + 1) * NLOC], zt[ch][:])
                    nc.sync.dma_start(cc_in[ch * 128:(ch + 1) * 128, :],
                                      zb_new[:, ch * NLOC:(ch + 1) * NLOC])
                    nc.scalar.activation(z2[ch][:], zt[ch][:], SQUARE)
                zb = zb_new

                # sq in column layout [128, 4] (for payload -> bias)
                sqc_ps = psum.tile([128, NJB_LOC], F32, name=f"sqc_{t}",
                                   tag="aux", bufs=2)
                for ib in range(NJB_LOC):
                    for ch in (0, 1):
                        nc.tensor.matmul(sqc_ps[:, ib:ib + 1],
                                         z2[ch][:, ib * 128:(ib + 1) * 128],
                                         onec[:],
                                         start=(ch == 0), stop=(ch == 1))
                sqc_b = work.tile([128, NJB_LOC], BF16, name=f"sqcb_{t}",
                                  tag="sqcb", bufs=2)
                nc.vector.tensor_copy(sqc_b[:], sqc_ps[:])
                nc.sync.dma_start(
                    cc_in[D:D + 2, :].rearrange("o (q b) -> (o q) b", b=8),
                    sqc_b[:].bitcast(F8))
                nc.gpsimd.collective_compute(
                    "AllGather", mybir.AluOpType.bypass,
                    replica_groups=[list(range(N_CORES))],
                    ins=[cc_in[:].opt()], outs=[cc_out[:].opt()],
                )

                # sq in row layout [1, 512] -> E row for next step
                # (off the doorbell critical path)
                sqr_ps = psum.tile([1, NLOC], F32, name=f"sqr_{t}",
                                   tag="aux", bufs=2)
                for ch in (0, 1):
                    nc.tensor.matmul(sqr_ps[:], onec[:], z2[ch][:],
                                     start=(ch == 0), stop=(ch == 1))
                e_row_new = state.tile([1, NLOC], BF16, name=f"er_{t}",
                                       tag="e_row")
                nc.scalar.activation(e_row_new[:], sqr_ps[:], EXP, scale=-CEXP,
                                     bias=lsa[:])
                e_row = e_row_new

                # ---- unpack gathered Z^T, sq (one DMA each) -------------
                cco3 = cc_out[:].rearrange("(r q) i -> r q i", r=N_CORES)
                HR = N_CORES // 2
                for half in (0, 1):
                    for ch in (0, 1):
                        nc.sync.dma_start(
                            ztf[:, ch * N + half * HR * NLOC:
                                ch * N + (half + 1) * HR * NLOC]
                            .rearrange("p (r i) -> p r i", r=HR),
                            cco3[half * HR:(half + 1) * HR,
                                 ch * 128:(ch + 1) * 128, :]
                            .rearrange("r p i -> p r i"))
                nc.sync.dma_start(
                    sqc_all[:].bitcast(F8).rearrange("p (r b) -> p r b",
                                                     r=N_CORES),
                    cco3[:, D:D + 2, :]
                    .rearrange("r o (q b) -> (o q) r b", b=8))
                # bias = -c * sq  (fp32, per-partition columns)
                nc.vector.tensor_scalar_mul(bias_col[:], sqc_all[:], -CEXP)
                nc.scalar.activation(e_col[:], bias_col[:], EXP)

                # next step's E broadcast, repeated FILLER_MMS times: the
                # matmul is idempotent (start=True each time), and the
                # repetitions keep TensorE's HAM clock-gate warm while the
                # AllGather + unpack are in flight.
                for _f in range(FILLER_MMS):
                    nc.tensor.matmul(e_ps_next[:], oner[:, 0:128], e_row[:],
                                     start=True, stop=True)
                e_sb = work.tile([128, NLOC], F32, name=f"e_sb_{t + 1}",
                                 tag="e_sb", bufs=2)
                nc.vector.tensor_copy(e_sb[:], e_ps_next[:])

    nc.compile()
    return nc


def _prepare_in_maps(X, A, A_aff, b_aff):
    XT = np.ascontiguousarray(X.T.astype(np.float32))          # [D, N]
    sq0 = (X.astype(np.float32) ** 2).sum(axis=1)              # [N]
    ztb_full0 = XT.astype(F8NP)
    bias_col0 = np.ascontiguousarray(
        (-CEXP * sq0).reshape(NJB, 128).T.astype(np.float32))  # [128, 32]
    a_b = (DT * SA * A.astype(np.float32)).astype(F8NP)        # [T, N, D]
    aaff_b = np.ascontiguousarray(
        (DT * SAF * A_aff.astype(np.float32)).transpose(0, 2, 1)).astype(F8NP)
    b_b = (DT * SAF * b_aff.astype(np.float32)).reshape(T, 1, D).astype(BF16NP)
    ones_col = np.ones((128, 1), dtype=BF16NP)
    ones_row = np.ones((1, NLOC), dtype=BF16NP)

    in_maps = []
    for c in range(N_CORES):
        cols = slice(c * NLOC, (c + 1) * NLOC)
        zt_local0 = np.ascontiguousarray(XT[:, cols])
        in_maps.append({
            "zt_local0": zt_local0,
            "zb_local0": zt_local0.astype(F8NP),
            "ztb_full0": ztb_full0,
            "bias_col0": bias_col0,
            "e_row0": (np.exp(-CEXP * sq0[cols]) / SA)[None, :].astype(BF16NP),
            "a_b": a_b,
            "aaff_b": aaff_b,
            "b_b": b_b,
            "ones_col": ones_col,
            "ones_row": ones_row,
            "log_inv_sa": np.array([[np.log(1.0 / SA)]], dtype=np.float32),
        })
    return in_maps


def _get_nc():
    if "nc" not in _CACHED:
        _CACHED["nc"] = _build()
    return _CACHED["nc"]


def kernel(X, A, A_aff, b_aff):
    X = np.asarray(X)
    A = np.asarray(A)
    A_aff = np.asarray(A_aff)
    b_aff = np.asarray(b_aff)
    nc = _get_nc()
    in_maps = _prepare_in_maps(X, A, A_aff, b_aff)
    res = bass_utils.run_bass_kernel_spmd(
        nc, in_maps, core_ids=list(range(N_CORES)))
    out = np.empty((N, D), dtype=np.float32)
    for c in range(N_CORES):
        out[c * NLOC:(c + 1) * NLOC, :] = res.results[c]["out_zt"].T
    return out



# revision 27
# speedup vs baseline: 22.3227x; 1.0002x over previous
"""Trainium2 Bass kernel for DiffeomorphicLearner (gnn_message_passing).

Math (per step t, T=8 steps):
    sq_i  = ||z_i||^2
    K_ij  = exp((2 z_i.z_j - sq_i - sq_j) / (2 rho^2))
    v     = Z @ Aaff_t.T + b_t + K @ A_t
    Z    <- Z + DT * v

Distribution: row-parallel over N=4096 across 8 cores (512 rows each).
Each core keeps its Z rows (fp32 master, stored TRANSPOSED as [D, n_loc])
and computes K^T slices [j, i_loc] against an all-gathered bf16 copy of
the full Z^T plus column-layout sq. One bf16 AllGather of [Z^T; sq]
(257 x 512 per rank) per step.

Precision: all matmuls bf16 in / fp32 PSUM accumulate; exp argument and
state updates fp32. sq_j enters as a per-partition ACT bias; the
exp(-c*sq_i) factor is factored out per-column and applied after the
K@A contraction (it is constant along j).

Scheduling: the j-block loop is software-pipelined with a skew — the
S = Z_j.Z_i matmuls and the exp run PIPE_LAG blocks ahead of the K@A
consumer matmuls, so the TensorE stream never stalls on ScalarE.
Bulk DRAM traffic (A_t stream, AllGather unpack) is issued as single
strided DMA instructions to avoid Sync-engine issue serialization.
"""

import numpy as np
import ml_dtypes

import concourse.bass as bass
import concourse.tile as tile
from concourse import bacc, mybir
from concourse import bass_utils

BF16NP = ml_dtypes.bfloat16
F8NP = ml_dtypes.float8_e4m3

N_CORES = 8
N, D, T = 4096, 256, 8
RHO = 16.0
DT = 1.0 / T
CEXP = 1.0 / (2.0 * RHO * RHO)  # 1/512
SA = 64.0                      # fp8 scale for A (values are subnormal otherwise)
SAF = 64.0                     # fp8 scale for A_aff / b_aff

NLOC = N // N_CORES            # 512 rows per core
NJB = N // 128                 # 32 j-blocks of 128
NJB_LOC = NLOC // 128          # 4 local i-blocks
PAY_R = D + 2                  # payload rows: 256 fp8 Z rows + 2 rows of bf16 sq bits
PIPE_LAG = 2                   # block-PAIR skew between S/exp and K@A matmuls
FILLER_MMS = 1                 # E-broadcast repetitions in the AG window
PRE_FILLERS = 1                # E-broadcast repetitions before the update

F32 = mybir.dt.float32
BF16 = mybir.dt.bfloat16
F8 = mybir.dt.float8e4
DR = mybir.MatmulPerfMode.DoubleRow

_CACHED = {}


def _build():
    """Build the 8-core SPMD Bass program (same program on every core)."""
    nc = bacc.Bacc("TRN2", target_bir_lowering=False, debug=False,
                   num_devices=N_CORES)

    # ---- DRAM I/O -------------------------------------------------------
    zt_local0 = nc.dram_tensor("zt_local0", [D, NLOC], F32, kind="ExternalInput")
    zb_local0 = nc.dram_tensor("zb_local0", [D, NLOC], F8, kind="ExternalInput")
    ztb_full0 = nc.dram_tensor("ztb_full0", [D, N], F8, kind="ExternalInput")
    bias_col0 = nc.dram_tensor("bias_col0", [128, NJB], F32, kind="ExternalInput")
    e_row0 = nc.dram_tensor("e_row0", [1, NLOC], BF16, kind="ExternalInput")
    a_b = nc.dram_tensor("a_b", [T, N, D], F8, kind="ExternalInput")
    aaff_b = nc.dram_tensor("aaff_b", [T, D, D], F8, kind="ExternalInput")
    b_b = nc.dram_tensor("b_b", [T, 1, D], BF16, kind="ExternalInput")
    ones_col = nc.dram_tensor("ones_col", [128, 1], BF16, kind="ExternalInput")
    ones_row = nc.dram_tensor("ones_row", [1, NLOC], BF16, kind="ExternalInput")
    log_inv_sa = nc.dram_tensor("log_inv_sa", [1, 1], F32, kind="ExternalInput")
    out_zt = nc.dram_tensor("out_zt", [D, NLOC], F32, kind="ExternalOutput")

    EXP = mybir.ActivationFunctionType.Exp
    SQUARE = mybir.ActivationFunctionType.Square
    COPY = mybir.ActivationFunctionType.Copy

    with tile.TileContext(nc) as tc:
        with tc.tile_pool(name="persist", bufs=1) as persist, \
             tc.tile_pool(name="state", bufs=2) as state, \
             tc.tile_pool(name="astream", bufs=2) as astream, \
             tc.tile_pool(name="kpool", bufs=6) as kpool, \
             tc.tile_pool(name="work", bufs=2) as work, \
             tc.tile_pool(name="psum", bufs=1, space="PSUM") as psum, \
             tc.tile_pool(name="dram", bufs=2, space="DRAM") as dram:

            # ---- constants / persistent buffers -------------------------
            onec = persist.tile([128, 1], BF16, name="onec")
            nc.sync.dma_start(onec[:], ones_col[:])
            oner = persist.tile([1, NLOC], BF16, name="oner")
            nc.sync.dma_start(oner[:], ones_row[:])
            lsa = persist.tile([1, 1], F32, name="lsa")
            nc.sync.dma_start(lsa[:], log_inv_sa[:])

            # full gathered Z^T (bf16), rewritten each step by unpack DMAs
            ztf = persist.tile([128, 2 * N], F8, name="ztf")
            for ch in (0, 1):
                nc.sync.dma_start(ztf[:, ch * N:(ch + 1) * N],
                                  ztb_full0[ch * 128:(ch + 1) * 128, :])

            # gathered sq in column layout -> prescaled ACT bias (-c * sq)
            bias_col = persist.tile([128, NJB], F32, name="bias_col")
            nc.sync.dma_start(bias_col[:], bias_col0[:])
            sqc_all = persist.tile([128, NJB], BF16, name="sqc_all")

            # local state: fp32 master + bf16 working copy
            zt = [state.tile([128, NLOC], F32, name=f"zt{ch}", tag=f"zt{ch}")
                  for ch in (0, 1)]
            for ch in (0, 1):
                nc.sync.dma_start(zt[ch][:], zt_local0[ch * 128:(ch + 1) * 128, :])
            zb = state.tile([128, 2 * NLOC], F8, name="zb", tag="zb")
            for ch in (0, 1):
                nc.sync.dma_start(zb[:, ch * NLOC:(ch + 1) * NLOC],
                                  zb_local0[ch * 128:(ch + 1) * 128, :])

            e_row = state.tile([1, NLOC], BF16, name="e_row", tag="e_row")
            nc.sync.dma_start(e_row[:], e_row0[:])

            # e_col = exp(-c*sq_j) in column layout (bias_col is -c*sq)
            e_col = persist.tile([128, NJB], F32, name="e_col")
            nc.scalar.activation(e_col[:], bias_col[:], EXP)

            # warm-up AllGather: absorbs the ~90us first-collective setup
            # concurrently with step-0 compute (nothing consumes its output)
            cc_warm_in = dram.tile([1, NLOC], BF16, name="cc_warm_in",
                                   bufs=1)
            cc_warm_out = dram.tile([N_CORES, NLOC], BF16, name="cc_warm_out",
                                    bufs=1, addr_space="Shared")
            nc.sync.dma_start(cc_warm_in[:], oner[:])
            nc.gpsimd.collective_compute(
                "AllGather", mybir.AluOpType.bypass,
                replica_groups=[list(range(N_CORES))],
                ins=[cc_warm_in[:].opt()], outs=[cc_warm_out[:].opt()],
            )

            for t in range(T):
                last = (t == T - 1)

                # ---- A_t: one 2 MB DMA into [128, 32*256] ---------------
                a_sb = astream.tile([128, NJB * D], F8, name=f"a_{t}", tag="a")
                nc.sync.dma_start(
                    a_sb[:].rearrange("p (j d) -> p j d", j=NJB),
                    a_b.ap()[t].rearrange("(j p) d -> p j d", p=128))

                # Aaff_t: one DMA into [128, 2*256]; b_t row
                aaff_sb = astream.tile([128, 2 * D], F8, name=f"aaff_{t}",
                                       tag="aaff")
                nc.sync.dma_start(
                    aaff_sb[:].rearrange("p (c d) -> p c d", c=2),
                    aaff_b.ap()[t].rearrange("(c p) d -> p c d", p=128))
                brow_t = astream.tile([1, D], BF16, name=f"brow_{t}", tag="brow")
                nc.sync.dma_start(brow_t[:], b_b.ap()[t, :, :])

                # ---- E broadcast: E[p, i] = exp(-c*sq_i) ----------------
                # (steps >= 1 build e_sb in the previous step's tail, where
                # the repeated idempotent matmul doubles as HAM keep-warm
                # filler during the AllGather wait)
                if t == 0:
                    e_ps = psum.tile([128, NLOC], F32, name="e_ps_0",
                                     tag="aux", bufs=2)
                    nc.tensor.matmul(e_ps[:], oner[:, 0:128], e_row[:],
                                     start=True, stop=True)
                    e_sb = work.tile([128, NLOC], F32, name="e_sb_0",
                                     tag="e_sb", bufs=2)
                    nc.vector.tensor_copy(e_sb[:], e_ps[:])

                # ---- affine part: va[dh] = Aaff_t @ z_loc + b_t ---------
                va_sb = []
                aaff3 = aaff_sb[:].rearrange("k (r d) -> k r d", r=2)
                zb3a = zb[:].rearrange("k (r i) -> k r i", r=2)
                for dh in (0, 1):
                    va = psum.tile([128, NLOC], F32, name=f"va_{t}_{dh}",
                                   tag="aux", bufs=2)
                    nc.tensor.matmul(va[:],
                                     aaff3[:, :, dh * 128:(dh + 1) * 128],
                                     zb3a[:], start=True, stop=False,
                                     perf_mode=DR)
                    nc.tensor.matmul(va[:],
                                     brow_t[:, dh * 128:(dh + 1) * 128],
                                     oner[:], start=False, stop=True)
                    # va holds SAF*(Aaff@z + b): descale, then add z master
                    vs0 = work.tile([128, NLOC], F32, name=f"vs0_{t}_{dh}",
                                    tag=f"vs0{dh}", bufs=2)
                    nc.vector.tensor_scalar_mul(vs0[:], va[:], 1.0 / SAF)
                    vsb = work.tile([128, NLOC], F32, name=f"vasb_{t}_{dh}",
                                    tag=f"vasb{dh}", bufs=2)
                    nc.vector.tensor_add(vsb[:], vs0[:], zt[dh][:])
                    va_sb.append(vsb)

                # ---- A' = A * exp(-c*sq_j): per-partition rescale -------
                a_sc = astream.tile([128, NJB * D], F8, name=f"asc_{t}",
                                    tag="asc")
                for jb in range(NJB):
                    nc.vector.tensor_scalar_mul(
                        a_sc[:, jb * D:(jb + 1) * D],
                        a_sb[:, jb * D:(jb + 1) * D],
                        e_col[:, jb:jb + 1])

                # ---- main loop over block PAIRS (DoubleRow, fp8) --------
                NPAIR = NJB // 2
                ztf3 = ztf[:].rearrange("k (r j) -> k r j", r=2)
                zb3 = zb[:].rearrange("k (r i) -> k r i", r=2)
                vr = [psum.tile([128, NLOC], F32, name=f"vr_{t}_{dh}",
                                tag=f"vr{dh}", bufs=1) for dh in (0, 1)]
                k_pairs = [None] * NPAIR
                for qq in range(NPAIR + PIPE_LAG):
                    if qq < NPAIR:
                        k_p = kpool.tile([128, 2 * NLOC], F8,
                                         name=f"k_{t}_{qq}", tag="k")
                        s_ps = psum.tile([128, 2 * NLOC], F32,
                                         name=f"s_{t}_{qq}",
                                         tag="s", bufs=2)
                        for h in (0, 1):
                            jb = 2 * qq + h
                            nc.tensor.matmul(
                                s_ps[:, h * NLOC:(h + 1) * NLOC],
                                ztf3[:, :, jb * 128:(jb + 1) * 128],
                                zb3[:], start=True, stop=True,
                                perf_mode=DR)
                        # G = exp(2cS); exp(-c sq_j) is pre-folded into A,
                        # exp(-c sq_i) post-multiplies vr
                        nc.scalar.activation(k_p[:], s_ps[:],
                                             EXP, scale=2.0 * CEXP)
                        k_pairs[qq] = k_p
                    if qq >= PIPE_LAG:
                        q = qq - PIPE_LAG
                        k_p = k_pairs[q]
                        a3 = a_sc[:, 2 * q * D:2 * (q + 1) * D].rearrange(
                            "k (r d) -> k r d", r=2)
                        k3 = k_p[:].rearrange("k (r i) -> k r i", r=2)
                        for dh in (0, 1):
                            nc.tensor.matmul(
                                vr[dh][:],
                                a3[:, :, dh * 128:(dh + 1) * 128],
                                k3[:],
                                start=(q == 0),
                                stop=(q == NPAIR - 1 and dh == 1),
                                perf_mode=DR)

                # keep TensorE busy across the DVE update window (the
                # clock-gate re-throttles on any PE idle) — idempotent,
                # overwritten by the real fillers in the tail
                if not last:
                    e_ps_next = psum.tile([128, NLOC], F32,
                                          name=f"e_ps_{t + 1}",
                                          tag="aux", bufs=2)
                    for _f in range(PRE_FILLERS):
                        nc.tensor.matmul(e_ps_next[:], oner[:, 0:128],
                                         e_row[:], start=True, stop=True)

                # ---- update: z <- z + va + vr * E -----------------------
                zt_new = [state.tile([128, NLOC], F32, name=f"ztn_{t}_{ch}",
                                     tag=f"zt{ch}") for ch in (0, 1)]
                for dh in (0, 1):
                    t1 = work.tile([128, NLOC], F32, name=f"t1_{t}_{dh}",
                                   tag="t1", bufs=2)
                    nc.vector.tensor_mul(t1[:], vr[dh][:], e_sb[:])
                    nc.vector.tensor_add(zt_new[dh][:], t1[:], va_sb[dh][:])
                zt = zt_new

                if last:
                    for ch in (0, 1):
                        nc.sync.dma_start(
                            out_zt[ch * 128:(ch + 1) * 128, :], zt[ch][:])
                    break

                # ---- post-update tail: bf16 copy, sq, payload, AG -------
                zb_new = state.tile([128, 2 * NLOC], F8, name=f"zbn_{t}",
                                    tag="zb")
                z2 = [work.tile([128, NLOC], BF16, name=f"z2_{t}_{ch}",
                                tag=f"z2{ch}", bufs=2) for ch in (0, 1)]
                for ch in (0, 1):
                    nc.vector.tensor_copy(
                        zb_new[:, ch * NLOC:(ch + 1) * NLOC], zt[ch][:])
                    nc.scalar.activation(z2[ch][:], zt[ch][:], SQUARE)
                zb = zb_new

                # sq in column layout [128, 4] (for payload -> bias)
                sqc_ps = psum.tile([128, NJB_LOC], F32, name=f"sqc_{t}",
                                   tag="aux", bufs=2)
                for ib in range(NJB_LOC):
                    for ch in (0, 1):
                        nc.tensor.matmul(sqc_ps[:, ib:ib + 1],
                                         z2[ch][:, ib * 128:(ib + 1) * 128],
                                         onec[:],
                                         start=(ch == 0), stop=(ch == 1))
                sqc_b = work.tile([128, NJB_LOC], BF16, name=f"sqcb_{t}",
                                  tag="sqcb", bufs=2)
                nc.vector.tensor_copy(sqc_b[:], sqc_ps[:])

                # sq in row layout [1, 512] -> E row for next step
                sqr_ps = psum.tile([1, NLOC], F32, name=f"sqr_{t}",
                                   tag="aux", bufs=2)
                for ch in (0, 1):
                    nc.tensor.matmul(sqr_ps[:], onec[:], z2[ch][:],
                                     start=(ch == 0), stop=(ch == 1))
                e_row_new = state.tile([1, NLOC], BF16, name=f"er_{t}",
                                       tag="e_row")
                nc.scalar.activation(e_row_new[:], sqr_ps[:], EXP, scale=-CEXP,
                                     bias=lsa[:])
                e_row = e_row_new

                # ---- pack payload + AllGather ---------------------------
                cc_in = dram.tile([PAY_R, NLOC], F8, name=f"cci_{t}",
                                  tag="cci")
                cc_out = dram.tile([N_CORES * PAY_R, NLOC], F8,
                                   name=f"cco_{t}", tag="cco",
                                   addr_space="Shared")
                for ch in (0, 1):
                    nc.sync.dma_start(cc_in[ch * 128:(ch + 1) * 128, :],
                                      zb[:, ch * NLOC:(ch + 1) * NLOC])
                nc.sync.dma_start(
                    cc_in[D:D + 2, :].rearrange("o (q b) -> (o q) b", b=8),
                    sqc_b[:].bitcast(F8))
                nc.gpsimd.collective_compute(
                    "AllGather", mybir.AluOpType.bypass,
                    replica_groups=[list(range(N_CORES))],
                    ins=[cc_in[:].opt()], outs=[cc_out[:].opt()],
                )

                # ---- unpack gathered sq first, then Z^T -----------------
                # (e_col -> a_sc -> first K@A is the longest post-AG chain,
                # so the tiny sq unpack goes ahead of the 1 MB Z unpack)
                cco3 = cc_out[:].rearrange("(r q) i -> r q i", r=N_CORES)
                HR = N_CORES // 2
                nc.sync.dma_start(
                    sqc_all[:].bitcast(F8).rearrange("p (r b) -> p r b",
                                                     r=N_CORES),
                    cco3[:, D:D + 2, :]
                    .rearrange("r o (q b) -> (o q) r b", b=8))
                # bias = -c * sq  (fp32, per-partition columns)
                nc.vector.tensor_scalar_mul(bias_col[:], sqc_all[:], -CEXP)
                nc.scalar.activation(e_col[:], bias_col[:], EXP)
                for half in (0, 1):
                    for ch in (0, 1):
                        nc.sync.dma_start(
                            ztf[:, ch * N + half * HR * NLOC:
                                ch * N + (half + 1) * HR * NLOC]
                            .rearrange("p (r i) -> p r i", r=HR),
                            cco3[half * HR:(half + 1) * HR,
                                 ch * 128:(ch + 1) * 128, :]
                            .rearrange("r p i -> p r i"))

                # next step's E broadcast, repeated FILLER_MMS times: the
                # matmul is idempotent (start=True each time), and the
                # repetitions keep TensorE's HAM clock-gate warm while the
                # AllGather + unpack are in flight.
                for _f in range(FILLER_MMS):
                    nc.tensor.matmul(e_ps_next[:], oner[:, 0:128], e_row[:],
                                     start=True, stop=True)
                e_sb = work.tile([128, NLOC], F32, name=f"e_sb_{t + 1}",
                                 tag="e_sb", bufs=2)
                nc.vector.tensor_copy(e_sb[:], e_ps_next[:])

    nc.compile()
    return nc


def _prepare_in_maps(X, A, A_aff, b_aff):
    XT = np.ascontiguousarray(X.T.astype(np.float32))          # [D, N]
    sq0 = (X.astype(np.float32) ** 2).sum(axis=1)              # [N]
    ztb_full0 = XT.astype(F8NP)
    bias_col0 = np.ascontiguousarray(
        (-CEXP * sq0).reshape(NJB, 128).T.astype(np.float32))  # [128, 32]
    a_b = (DT * SA * A.astype(np.float32)).astype(F8NP)        # [T, N, D]
    aaff_b = np.ascontiguousarray(
        (DT * SAF * A_aff.astype(np.float32)).transpose(0, 2, 1)).astype(F8NP)
    b_b = (DT * SAF * b_aff.astype(np.float32)).reshape(T, 1, D).astype(BF16NP)
    ones_col = np.ones((128, 1), dtype=BF16NP)
    ones_row = np.ones((1, NLOC), dtype=BF16NP)

    in_maps = []
    for c in range(N_CORES):
        cols = slice(c * NLOC, (c + 1) * NLOC)
        zt_local0 = np.ascontiguousarray(XT[:, cols])
        in_maps.append({
            "zt_local0": zt_local0,
            "zb_local0": zt_local0.astype(F8NP),
            "ztb_full0": ztb_full0,
            "bias_col0": bias_col0,
            "e_row0": (np.exp(-CEXP * sq0[cols]) / SA)[None, :].astype(BF16NP),
            "a_b": a_b,
            "aaff_b": aaff_b,
            "b_b": b_b,
            "ones_col": ones_col,
            "ones_row": ones_row,
            "log_inv_sa": np.array([[np.log(1.0 / SA)]], dtype=np.float32),
        })
    return in_maps


def _get_nc():
    if "nc" not in _CACHED:
        _CACHED["nc"] = _build()
    return _CACHED["nc"]


def kernel(X, A, A_aff, b_aff):
    X = np.asarray(X)
    A = np.asarray(A)
    A_aff = np.asarray(A_aff)
    b_aff = np.asarray(b_aff)
    nc = _get_nc()
    in_maps = _prepare_in_maps(X, A, A_aff, b_aff)
    res = bass_utils.run_bass_kernel_spmd(
        nc, in_maps, core_ids=list(range(N_CORES)))
    out = np.empty((N, D), dtype=np.float32)
    for c in range(N_CORES):
        out[c * NLOC:(c + 1) * NLOC, :] = res.results[c]["out_zt"].T
    return out



# revision 28
# speedup vs baseline: 22.6529x; 1.0148x over previous
"""Trainium2 Bass kernel for DiffeomorphicLearner (gnn_message_passing).

Math (per step t, T=8 steps):
    sq_i  = ||z_i||^2
    K_ij  = exp((2 z_i.z_j - sq_i - sq_j) / (2 rho^2))
    v     = Z @ Aaff_t.T + b_t + K @ A_t
    Z    <- Z + DT * v

Distribution: row-parallel over N=4096 across 8 cores (512 rows each).
Each core keeps its Z rows (fp32 master, stored TRANSPOSED as [D, n_loc])
and computes K^T slices [j, i_loc] against an all-gathered bf16 copy of
the full Z^T plus column-layout sq. One bf16 AllGather of [Z^T; sq]
(257 x 512 per rank) per step.

Precision: all matmuls bf16 in / fp32 PSUM accumulate; exp argument and
state updates fp32. sq_j enters as a per-partition ACT bias; the
exp(-c*sq_i) factor is factored out per-column and applied after the
K@A contraction (it is constant along j).

Scheduling: the j-block loop is software-pipelined with a skew — the
S = Z_j.Z_i matmuls and the exp run PIPE_LAG blocks ahead of the K@A
consumer matmuls, so the TensorE stream never stalls on ScalarE.
Bulk DRAM traffic (A_t stream, AllGather unpack) is issued as single
strided DMA instructions to avoid Sync-engine issue serialization.
"""

import numpy as np
import ml_dtypes

import concourse.bass as bass
import concourse.tile as tile
from concourse import bacc, mybir
from concourse import bass_utils

BF16NP = ml_dtypes.bfloat16
F8NP = ml_dtypes.float8_e4m3

N_CORES = 8
N, D, T = 4096, 256, 8
RHO = 16.0
DT = 1.0 / T
CEXP = 1.0 / (2.0 * RHO * RHO)  # 1/512
SA = 64.0                      # fp8 scale for A (values are subnormal otherwise)
SAF = 64.0                     # fp8 scale for A_aff / b_aff

NLOC = N // N_CORES            # 512 rows per core
NJB = N // 128                 # 32 j-blocks of 128
NJB_LOC = NLOC // 128          # 4 local i-blocks
PAY_R = D + 2                  # payload rows: 256 fp8 Z rows + 2 rows of bf16 sq bits
PIPE_LAG = 2                   # block-PAIR skew between S/exp and K@A matmuls
FILLER_MMS = 1                 # E-broadcast repetitions in the AG window
PRE_FILLERS = 1                # E-broadcast repetitions before the update

F32 = mybir.dt.float32
BF16 = mybir.dt.bfloat16
F8 = mybir.dt.float8e4
DR = mybir.MatmulPerfMode.DoubleRow

_CACHED = {}


def _build():
    """Build the 8-core SPMD Bass program (same program on every core)."""
    nc = bacc.Bacc("TRN2", target_bir_lowering=False, debug=False,
                   num_devices=N_CORES)

    # ---- DRAM I/O -------------------------------------------------------
    zt_local0 = nc.dram_tensor("zt_local0", [D, NLOC], F32, kind="ExternalInput")
    zb_local0 = nc.dram_tensor("zb_local0", [D, NLOC], F8, kind="ExternalInput")
    ztb_full0 = nc.dram_tensor("ztb_full0", [D, N], F8, kind="ExternalInput")
    bias_col0 = nc.dram_tensor("bias_col0", [128, NJB], F32, kind="ExternalInput")
    e_row0 = nc.dram_tensor("e_row0", [1, NLOC], BF16, kind="ExternalInput")
    a_b = nc.dram_tensor("a_b", [T, N, D], F8, kind="ExternalInput")
    aaff_b = nc.dram_tensor("aaff_b", [T, D, D], F8, kind="ExternalInput")
    b_b = nc.dram_tensor("b_b", [T, 1, D], BF16, kind="ExternalInput")
    ones_col = nc.dram_tensor("ones_col", [128, 1], BF16, kind="ExternalInput")
    ones_row = nc.dram_tensor("ones_row", [1, NLOC], BF16, kind="ExternalInput")
    log_inv_sa = nc.dram_tensor("log_inv_sa", [1, 1], F32, kind="ExternalInput")
    out_zt = nc.dram_tensor("out_zt", [D, NLOC], F32, kind="ExternalOutput")

    EXP = mybir.ActivationFunctionType.Exp
    SQUARE = mybir.ActivationFunctionType.Square
    COPY = mybir.ActivationFunctionType.Copy

    with tile.TileContext(nc) as tc:
        with tc.tile_pool(name="persist", bufs=1) as persist, \
             tc.tile_pool(name="state", bufs=2) as state, \
             tc.tile_pool(name="astream", bufs=2) as astream, \
             tc.tile_pool(name="kpool", bufs=6) as kpool, \
             tc.tile_pool(name="work", bufs=2) as work, \
             tc.tile_pool(name="psum", bufs=1, space="PSUM") as psum, \
             tc.tile_pool(name="dram", bufs=2, space="DRAM") as dram:

            # ---- constants / persistent buffers -------------------------
            onec = persist.tile([128, 1], BF16, name="onec")
            nc.sync.dma_start(onec[:], ones_col[:])
            oner = persist.tile([1, NLOC], BF16, name="oner")
            nc.sync.dma_start(oner[:], ones_row[:])
            lsa = persist.tile([1, 1], F32, name="lsa")
            nc.sync.dma_start(lsa[:], log_inv_sa[:])

            # full gathered Z^T (bf16), rewritten each step by unpack DMAs
            ztf = persist.tile([128, 2 * N], F8, name="ztf")
            for ch in (0, 1):
                nc.sync.dma_start(ztf[:, ch * N:(ch + 1) * N],
                                  ztb_full0[ch * 128:(ch + 1) * 128, :])

            # gathered sq in column layout -> prescaled ACT bias (-c * sq)
            bias_col = persist.tile([128, NJB], F32, name="bias_col")
            nc.sync.dma_start(bias_col[:], bias_col0[:])
            sqc_all = persist.tile([128, NJB], BF16, name="sqc_all")

            # local state: fp32 master + bf16 working copy
            zt = [state.tile([128, NLOC], F32, name=f"zt{ch}", tag=f"zt{ch}")
                  for ch in (0, 1)]
            for ch in (0, 1):
                nc.sync.dma_start(zt[ch][:], zt_local0[ch * 128:(ch + 1) * 128, :])
            zb = state.tile([128, 2 * NLOC], F8, name="zb", tag="zb")
            for ch in (0, 1):
                nc.sync.dma_start(zb[:, ch * NLOC:(ch + 1) * NLOC],
                                  zb_local0[ch * 128:(ch + 1) * 128, :])

            e_row = state.tile([1, NLOC], BF16, name="e_row", tag="e_row")
            nc.sync.dma_start(e_row[:], e_row0[:])

            # e_col = exp(-c*sq_j) in column layout (bias_col is -c*sq)
            e_col = persist.tile([128, NJB], F32, name="e_col")
            nc.scalar.activation(e_col[:], bias_col[:], EXP)

            # warm-up AllGather: absorbs the ~90us first-collective setup
            # concurrently with step-0 compute (nothing consumes its output)
            cc_warm_in = dram.tile([1, NLOC], BF16, name="cc_warm_in",
                                   bufs=1)
            cc_warm_out = dram.tile([N_CORES, NLOC], BF16, name="cc_warm_out",
                                    bufs=1, addr_space="Shared")
            nc.sync.dma_start(cc_warm_in[:], oner[:])
            nc.gpsimd.collective_compute(
                "AllGather", mybir.AluOpType.bypass,
                replica_groups=[list(range(N_CORES))],
                ins=[cc_warm_in[:].opt()], outs=[cc_warm_out[:].opt()],
            )

            for t in range(T):
                last = (t == T - 1)

                # ---- A_t: one 2 MB DMA into [128, 32*256] ---------------
                a_sb = astream.tile([128, NJB * D], F8, name=f"a_{t}", tag="a")
                nc.sync.dma_start(
                    a_sb[:].rearrange("p (j d) -> p j d", j=NJB),
                    a_b.ap()[t].rearrange("(j p) d -> p j d", p=128))

                # Aaff_t: one DMA into [128, 2*256]; b_t row
                aaff_sb = astream.tile([128, 2 * D], F8, name=f"aaff_{t}",
                                       tag="aaff")
                nc.sync.dma_start(
                    aaff_sb[:].rearrange("p (c d) -> p c d", c=2),
                    aaff_b.ap()[t].rearrange("(c p) d -> p c d", p=128))
                brow_t = astream.tile([1, D], BF16, name=f"brow_{t}", tag="brow")
                nc.sync.dma_start(brow_t[:], b_b.ap()[t, :, :])

                # ---- E broadcast: E[p, i] = exp(-c*sq_i) ----------------
                # (steps >= 1 build e_sb in the previous step's tail, where
                # the repeated idempotent matmul doubles as HAM keep-warm
                # filler during the AllGather wait)
                if t == 0:
                    e_ps = psum.tile([128, NLOC], F32, name="e_ps_0",
                                     tag="aux", bufs=2)
                    nc.tensor.matmul(e_ps[:], oner[:, 0:128], e_row[:],
                                     start=True, stop=True)
                    e_sb = work.tile([128, NLOC], F32, name="e_sb_0",
                                     tag="e_sb", bufs=2)
                    nc.vector.tensor_copy(e_sb[:], e_ps[:])

                # ---- affine part: va[dh] = Aaff_t @ z_loc + b_t ---------
                va_sb = []
                aaff3 = aaff_sb[:].rearrange("k (r d) -> k r d", r=2)
                zb3a = zb[:].rearrange("k (r i) -> k r i", r=2)
                for dh in (0, 1):
                    va = psum.tile([128, NLOC], F32, name=f"va_{t}_{dh}",
                                   tag="aux", bufs=2)
                    nc.tensor.matmul(va[:],
                                     aaff3[:, :, dh * 128:(dh + 1) * 128],
                                     zb3a[:], start=True, stop=False,
                                     perf_mode=DR)
                    nc.tensor.matmul(va[:],
                                     brow_t[:, dh * 128:(dh + 1) * 128],
                                     oner[:], start=False, stop=True)
                    # va holds SAF*(Aaff@z + b): descale, then add z master
                    vs0 = work.tile([128, NLOC], F32, name=f"vs0_{t}_{dh}",
                                    tag=f"vs0{dh}", bufs=2)
                    nc.vector.tensor_scalar_mul(vs0[:], va[:], 1.0 / SAF)
                    vsb = work.tile([128, NLOC], F32, name=f"vasb_{t}_{dh}",
                                    tag=f"vasb{dh}", bufs=2)
                    nc.vector.tensor_add(vsb[:], vs0[:], zt[dh][:])
                    va_sb.append(vsb)

                # ---- A' = A * exp(-c*sq_j): per-partition rescale -------
                a_sc = astream.tile([128, NJB * D], F8, name=f"asc_{t}",
                                    tag="asc")
                for jb in range(NJB):
                    nc.vector.tensor_scalar_mul(
                        a_sc[:, jb * D:(jb + 1) * D],
                        a_sb[:, jb * D:(jb + 1) * D],
                        e_col[:, jb:jb + 1])

                # ---- main loop over block PAIRS (DoubleRow, fp8) --------
                NPAIR = NJB // 2
                ztf3 = ztf[:].rearrange("k (r j) -> k r j", r=2)
                zb3 = zb[:].rearrange("k (r i) -> k r i", r=2)
                vr = [psum.tile([128, NLOC], F32, name=f"vr_{t}_{dh}",
                                tag=f"vr{dh}", bufs=1) for dh in (0, 1)]
                k_pairs = [None] * NPAIR
                for qq in range(NPAIR + PIPE_LAG):
                    if qq < NPAIR:
                        k_p = kpool.tile([128, 2 * NLOC], F8,
                                         name=f"k_{t}_{qq}", tag="k")
                        s_ps = psum.tile([128, 2 * NLOC], F32,
                                         name=f"s_{t}_{qq}",
                                         tag="s", bufs=2)
                        for h in (0, 1):
                            jb = 2 * qq + h
                            nc.tensor.matmul(
                                s_ps[:, h * NLOC:(h + 1) * NLOC],
                                ztf3[:, :, jb * 128:(jb + 1) * 128],
                                zb3[:], start=True, stop=True,
                                perf_mode=DR)
                        # G = exp(2cS); exp(-c sq_j) is pre-folded into A,
                        # exp(-c sq_i) post-multiplies vr
                        nc.scalar.activation(k_p[:], s_ps[:],
                                             EXP, scale=2.0 * CEXP)
                        k_pairs[qq] = k_p
                    if qq >= PIPE_LAG:
                        q = qq - PIPE_LAG
                        k_p = k_pairs[q]
                        a3 = a_sc[:, 2 * q * D:2 * (q + 1) * D].rearrange(
                            "k (r d) -> k r d", r=2)
                        k3 = k_p[:].rearrange("k (r i) -> k r i", r=2)
                        for dh in (0, 1):
                            nc.tensor.matmul(
                                vr[dh][:],
                                a3[:, :, dh * 128:(dh + 1) * 128],
                                k3[:],
                                start=(q == 0),
                                stop=(q == NPAIR - 1 and dh == 1),
                                perf_mode=DR)

                # keep TensorE busy across the DVE update window (the
                # clock-gate re-throttles on any PE idle) — idempotent,
                # overwritten by the real fillers in the tail
                if not last:
                    e_ps_next = psum.tile([128, NLOC], F32,
                                          name=f"e_ps_{t + 1}",
                                          tag="aux", bufs=2)
                    for _f in range(PRE_FILLERS):
                        nc.tensor.matmul(e_ps_next[:], oner[:, 0:128],
                                         e_row[:], start=True, stop=True)

                # ---- update: z <- z + va + vr * E -----------------------
                zt_new = [state.tile([128, NLOC], F32, name=f"ztn_{t}_{ch}",
                                     tag=f"zt{ch}") for ch in (0, 1)]
                for dh in (0, 1):
                    t1 = work.tile([128, NLOC], F32, name=f"t1_{t}_{dh}",
                                   tag="t1", bufs=2)
                    nc.vector.tensor_mul(t1[:], vr[dh][:], e_sb[:])
                    nc.vector.tensor_add(zt_new[dh][:], t1[:], va_sb[dh][:])
                zt = zt_new

                if last:
                    for ch in (0, 1):
                        nc.sync.dma_start(
                            out_zt[ch * 128:(ch + 1) * 128, :], zt[ch][:])
                    break

                # ---- post-update tail: bf16 copy, sq, payload, AG -------
                zb_new = state.tile([128, 2 * NLOC], F8, name=f"zbn_{t}",
                                    tag="zb")
                z2 = [work.tile([128, NLOC], BF16, name=f"z2_{t}_{ch}",
                                tag=f"z2{ch}", bufs=2) for ch in (0, 1)]
                for ch in (0, 1):
                    nc.vector.tensor_copy(
                        zb_new[:, ch * NLOC:(ch + 1) * NLOC], zt[ch][:])
                    nc.scalar.activation(z2[ch][:], zt[ch][:], SQUARE)
                zb = zb_new

                # sq in column layout [128, 4] (for payload -> bias)
                sqc_ps = psum.tile([128, NJB_LOC], F32, name=f"sqc_{t}",
                                   tag="aux", bufs=2)
                for ib in range(NJB_LOC):
                    for ch in (0, 1):
                        nc.tensor.matmul(sqc_ps[:, ib:ib + 1],
                                         z2[ch][:, ib * 128:(ib + 1) * 128],
                                         onec[:],
                                         start=(ch == 0), stop=(ch == 1))
                sqc_b = work.tile([128, NJB_LOC], BF16, name=f"sqcb_{t}",
                                  tag="sqcb", bufs=2)
                nc.vector.tensor_copy(sqc_b[:], sqc_ps[:])

                # sq in row layout [1, 512] -> E row for next step
                sqr_ps = psum.tile([1, NLOC], F32, name=f"sqr_{t}",
                                   tag="aux", bufs=2)
                for ch in (0, 1):
                    nc.tensor.matmul(sqr_ps[:], onec[:], z2[ch][:],
                                     start=(ch == 0), stop=(ch == 1))
                e_row_new = state.tile([1, NLOC], BF16, name=f"er_{t}",
                                       tag="e_row")
                nc.scalar.activation(e_row_new[:], sqr_ps[:], EXP, scale=-CEXP,
                                     bias=lsa[:])
                e_row = e_row_new

                # ---- pack payload + AllGather ---------------------------
                cc_in = dram.tile([PAY_R, NLOC], F8, name=f"cci_{t}",
                                  tag="cci")
                cc_out = dram.tile([N_CORES * PAY_R, NLOC], F8,
                                   name=f"cco_{t}", tag="cco",
                                   addr_space="Shared")
                for ch in (0, 1):
                    nc.sync.dma_start(cc_in[ch * 128:(ch + 1) * 128, :],
                                      zb[:, ch * NLOC:(ch + 1) * NLOC])
                nc.sync.dma_start(
                    cc_in[D:D + 2, :].rearrange("o (q b) -> (o q) b", b=8),
                    sqc_b[:].bitcast(F8))
                nc.gpsimd.collective_compute(
                    "AllGather", mybir.AluOpType.bypass,
                    replica_groups=[list(range(N_CORES))],
                    ins=[cc_in[:].opt()], outs=[cc_out[:].opt()],
                )

                # ---- unpack gathered Z^T, sq (one DMA each) -------------
                cco3 = cc_out[:].rearrange("(r q) i -> r q i", r=N_CORES)
                HR = N_CORES // 2
                for half in (0, 1):
                    for ch in (0, 1):
                        nc.sync.dma_start(
                            ztf[:, ch * N + half * HR * NLOC:
                                ch * N + (half + 1) * HR * NLOC]
                            .rearrange("p (r i) -> p r i", r=HR),
                            cco3[half * HR:(half + 1) * HR,
                                 ch * 128:(ch + 1) * 128, :]
                            .rearrange("r p i -> p r i"))
                nc.sync.dma_start(
                    sqc_all[:].bitcast(F8).rearrange("p (r b) -> p r b",
                                                     r=N_CORES),
                    cco3[:, D:D + 2, :]
                    .rearrange("r o (q b) -> (o q) r b", b=8))
                # bias = -c * sq  (fp32, per-partition columns)
                nc.vector.tensor_scalar_mul(bias_col[:], sqc_all[:], -CEXP)
                nc.scalar.activation(e_col[:], bias_col[:], EXP)

                # next step's E broadcast, repeated FILLER_MMS times: the
                # matmul is idempotent (start=True each time), and the
                # repetitions keep TensorE's HAM clock-gate warm while the
                # AllGather + unpack are in flight.
                for _f in range(FILLER_MMS):
                    nc.tensor.matmul(e_ps_next[:], oner[:, 0:128], e_row[:],
                                     start=True, stop=True)
                e_sb = work.tile([128, NLOC], F32, name=f"e_sb_{t + 1}",
                                 tag="e_sb", bufs=2)
                nc.vector.tensor_copy(e_sb[:], e_ps_next[:])

    nc.compile()
    return nc


def _prepare_in_maps(X, A, A_aff, b_aff):
    XT = np.ascontiguousarray(X.T.astype(np.float32))          # [D, N]
    sq0 = (X.astype(np.float32) ** 2).sum(axis=1)              # [N]
    ztb_full0 = XT.astype(F8NP)
    bias_col0 = np.ascontiguousarray(
        (-CEXP * sq0).reshape(NJB, 128).T.astype(np.float32))  # [128, 32]
    a_b = (DT * SA * A.astype(np.float32)).astype(F8NP)        # [T, N, D]
    aaff_b = np.ascontiguousarray(
        (DT * SAF * A_aff.astype(np.float32)).transpose(0, 2, 1)).astype(F8NP)
    b_b = (DT * SAF * b_aff.astype(np.float32)).reshape(T, 1, D).astype(BF16NP)
    ones_col = np.ones((128, 1), dtype=BF16NP)
    ones_row = np.ones((1, NLOC), dtype=BF16NP)

    in_maps = []
    for c in range(N_CORES):
        cols = slice(c * NLOC, (c + 1) * NLOC)
        zt_local0 = np.ascontiguousarray(XT[:, cols])
        in_maps.append({
            "zt_local0": zt_local0,
            "zb_local0": zt_local0.astype(F8NP),
            "ztb_full0": ztb_full0,
            "bias_col0": bias_col0,
            "e_row0": (np.exp(-CEXP * sq0[cols]) / SA)[None, :].astype(BF16NP),
            "a_b": a_b,
            "aaff_b": aaff_b,
            "b_b": b_b,
            "ones_col": ones_col,
            "ones_row": ones_row,
            "log_inv_sa": np.array([[np.log(1.0 / SA)]], dtype=np.float32),
        })
    return in_maps


def _get_nc():
    if "nc" not in _CACHED:
        _CACHED["nc"] = _build()
    return _CACHED["nc"]


def kernel(X, A, A_aff, b_aff):
    X = np.asarray(X)
    A = np.asarray(A)
    A_aff = np.asarray(A_aff)
    b_aff = np.asarray(b_aff)
    nc = _get_nc()
    in_maps = _prepare_in_maps(X, A, A_aff, b_aff)
    res = bass_utils.run_bass_kernel_spmd(
        nc, in_maps, core_ids=list(range(N_CORES)))
    out = np.empty((N, D), dtype=np.float32)
    for c in range(N_CORES):
        out[c * NLOC:(c + 1) * NLOC, :] = res.results[c]["out_zt"].T
    return out



# revision 31
# speedup vs baseline: 25.0131x; 1.1042x over previous
"""Trainium2 Bass kernel for DiffeomorphicLearner (gnn_message_passing).

Math (per step t, T=8 steps):
    sq_i  = ||z_i||^2
    K_ij  = exp((2 z_i.z_j - sq_i - sq_j) / (2 rho^2))
    v     = Z @ Aaff_t.T + b_t + K @ A_t
    Z    <- Z + DT * v

Distribution: row-parallel over N=4096 across 8 cores (512 rows each).
Each core keeps its Z rows (fp32 master, stored TRANSPOSED as [D, n_loc])
and computes K^T slices [j, i_loc] against an all-gathered bf16 copy of
the full Z^T plus column-layout sq. One bf16 AllGather of [Z^T; sq]
(257 x 512 per rank) per step.

Precision: all matmuls bf16 in / fp32 PSUM accumulate; exp argument and
state updates fp32. sq_j enters as a per-partition ACT bias; the
exp(-c*sq_i) factor is factored out per-column and applied after the
K@A contraction (it is constant along j).

Scheduling: the j-block loop is software-pipelined with a skew — the
S = Z_j.Z_i matmuls and the exp run PIPE_LAG blocks ahead of the K@A
consumer matmuls, so the TensorE stream never stalls on ScalarE.
Bulk DRAM traffic (A_t stream, AllGather unpack) is issued as single
strided DMA instructions to avoid Sync-engine issue serialization.
"""

import numpy as np
import ml_dtypes

import concourse.bass as bass
import concourse.tile as tile
from concourse import bacc, mybir
from concourse import bass_utils

BF16NP = ml_dtypes.bfloat16
F8NP = ml_dtypes.float8_e4m3

N_CORES = 8
N, D, T = 4096, 256, 8
RHO = 16.0
DT = 1.0 / T
CEXP = 1.0 / (2.0 * RHO * RHO)  # 1/512
SA = 64.0                      # fp8 scale for A (values are subnormal otherwise)
SAF = 64.0                     # fp8 scale for A_aff / b_aff

NLOC = N // N_CORES            # 512 rows per core
NJB = N // 128                 # 32 j-blocks of 128
NJB_LOC = NLOC // 128          # 4 local i-blocks
PAY_R = D + 2                  # payload rows: 256 fp8 Z rows + 2 rows of bf16 sq bits
PIPE_LAG = 2                   # block-PAIR skew between S/exp and K@A matmuls
FILLER_MMS = 1                 # E-broadcast repetitions in the AG window
PRE_FILLERS = 1                # E-broadcast repetitions before the update

F32 = mybir.dt.float32
BF16 = mybir.dt.bfloat16
F8 = mybir.dt.float8e4
DR = mybir.MatmulPerfMode.DoubleRow

_CACHED = {}


def _build():
    """Build the 8-core SPMD Bass program (same program on every core)."""
    nc = bacc.Bacc("TRN2", target_bir_lowering=False, debug=False,
                   num_devices=N_CORES)

    # ---- DRAM I/O -------------------------------------------------------
    zt_local0 = nc.dram_tensor("zt_local0", [D, NLOC], F32, kind="ExternalInput")
    zb_local0 = nc.dram_tensor("zb_local0", [D, NLOC], F8, kind="ExternalInput")
    ztb_full0 = nc.dram_tensor("ztb_full0", [D, N], F8, kind="ExternalInput")
    bias_col0 = nc.dram_tensor("bias_col0", [128, NJB], F32, kind="ExternalInput")
    e_row0 = nc.dram_tensor("e_row0", [1, NLOC], BF16, kind="ExternalInput")
    a_b = nc.dram_tensor("a_b", [T, N, D], F8, kind="ExternalInput")
    aaff_b = nc.dram_tensor("aaff_b", [T, D, D], F8, kind="ExternalInput")
    b_b = nc.dram_tensor("b_b", [T, 1, D], BF16, kind="ExternalInput")
    ones_col = nc.dram_tensor("ones_col", [128, 1], BF16, kind="ExternalInput")
    ones_row = nc.dram_tensor("ones_row", [1, NLOC], BF16, kind="ExternalInput")
    log_inv_sa = nc.dram_tensor("log_inv_sa", [1, 1], F32, kind="ExternalInput")
    out_zt = nc.dram_tensor("out_zt", [D, NLOC], F32, kind="ExternalOutput")

    EXP = mybir.ActivationFunctionType.Exp
    SQUARE = mybir.ActivationFunctionType.Square
    COPY = mybir.ActivationFunctionType.Copy

    with tile.TileContext(nc) as tc:
        with tc.tile_pool(name="persist", bufs=1) as persist, \
             tc.tile_pool(name="state", bufs=2) as state, \
             tc.tile_pool(name="astream", bufs=2) as astream, \
             tc.tile_pool(name="kpool", bufs=6) as kpool, \
             tc.tile_pool(name="work", bufs=2) as work, \
             tc.tile_pool(name="psum", bufs=1, space="PSUM") as psum, \
             tc.tile_pool(name="dram", bufs=2, space="DRAM") as dram:

            # ---- constants / persistent buffers -------------------------
            onec = persist.tile([128, 1], BF16, name="onec")
            nc.sync.dma_start(onec[:], ones_col[:])
            oner = persist.tile([1, NLOC], BF16, name="oner")
            nc.sync.dma_start(oner[:], ones_row[:])
            lsa = persist.tile([1, 1], F32, name="lsa")
            nc.sync.dma_start(lsa[:], log_inv_sa[:])

            # full gathered Z^T (bf16), rewritten each step by unpack DMAs
            ztf = persist.tile([128, 2 * N], F8, name="ztf")
            for ch in (0, 1):
                nc.sync.dma_start(ztf[:, ch * N:(ch + 1) * N],
                                  ztb_full0[ch * 128:(ch + 1) * 128, :])

            # gathered sq in column layout -> prescaled ACT bias (-c * sq)
            bias_col = persist.tile([128, NJB], F32, name="bias_col")
            nc.sync.dma_start(bias_col[:], bias_col0[:])
            sqc_all = persist.tile([128, NJB], BF16, name="sqc_all")

            # local state: fp32 master + bf16 working copy
            zt = [state.tile([128, NLOC], F32, name=f"zt{ch}", tag=f"zt{ch}")
                  for ch in (0, 1)]
            for ch in (0, 1):
                nc.sync.dma_start(zt[ch][:], zt_local0[ch * 128:(ch + 1) * 128, :])
            zb = state.tile([128, 2 * NLOC], F8, name="zb", tag="zb")
            for ch in (0, 1):
                nc.sync.dma_start(zb[:, ch * NLOC:(ch + 1) * NLOC],
                                  zb_local0[ch * 128:(ch + 1) * 128, :])

            e_row = state.tile([1, NLOC], BF16, name="e_row", tag="e_row")
            nc.sync.dma_start(e_row[:], e_row0[:])

            # e_col = exp(-c*sq_j) in column layout (bias_col is -c*sq).
            # Two ping-ponged tiles, one step STALE: step t rescales A with
            # sq(t-1) instead of sq(t), so the rescale no longer gates on
            # this step's AllGather+unpack and runs inside the dead window.
            # sq drifts ~|2 z.dz| ~ 1.3/step -> K off by e^(+-c*1.3) ~ 0.25%,
            # ~1e-4/step of z drift -- far inside the 2e-2 gate.
            # Schedule: a_sc(t) reads e_col[t%2]; unpack(t) overwrites
            # e_col[t%2] with sq(t+1), which step t+2 wants.  Both tiles
            # start as exp(-c*sq(0)): exact for t=0, stale-by-one for t=1.
            e_cols = [persist.tile([128, NJB], F32, name=f"e_col{p}")
                      for p in (0, 1)]
            for p in (0, 1):
                nc.scalar.activation(e_cols[p][:], bias_col[:], EXP)

            # warm-up AllGather: absorbs the ~90us first-collective setup
            # concurrently with step-0 compute (nothing consumes its output)
            cc_warm_in = dram.tile([1, NLOC], BF16, name="cc_warm_in",
                                   bufs=1)
            cc_warm_out = dram.tile([N_CORES, NLOC], BF16, name="cc_warm_out",
                                    bufs=1, addr_space="Shared")
            nc.sync.dma_start(cc_warm_in[:], oner[:])
            nc.gpsimd.collective_compute(
                "AllGather", mybir.AluOpType.bypass,
                replica_groups=[list(range(N_CORES))],
                ins=[cc_warm_in[:].opt()], outs=[cc_warm_out[:].opt()],
            )

            for t in range(T):
                last = (t == T - 1)

                # ---- A_t: one 2 MB DMA into [128, 32*256] ---------------
                a_sb = astream.tile([128, NJB * D], F8, name=f"a_{t}", tag="a")
                nc.sync.dma_start(
                    a_sb[:].rearrange("p (j d) -> p j d", j=NJB),
                    a_b.ap()[t].rearrange("(j p) d -> p j d", p=128))

                # Aaff_t: one DMA into [128, 2*256]; b_t row
                aaff_sb = astream.tile([128, 2 * D], F8, name=f"aaff_{t}",
                                       tag="aaff")
                nc.sync.dma_start(
                    aaff_sb[:].rearrange("p (c d) -> p c d", c=2),
                    aaff_b.ap()[t].rearrange("(c p) d -> p c d", p=128))
                brow_t = astream.tile([1, D], BF16, name=f"brow_{t}", tag="brow")
                nc.sync.dma_start(brow_t[:], b_b.ap()[t, :, :])

                # ---- E broadcast: E[p, i] = exp(-c*sq_i) ----------------
                # (steps >= 1 build e_sb in the previous step's tail, where
                # the repeated idempotent matmul doubles as HAM keep-warm
                # filler during the AllGather wait)
                if t == 0:
                    e_ps = psum.tile([128, NLOC], F32, name="e_ps_0",
                                     tag="aux", bufs=2)
                    nc.tensor.matmul(e_ps[:], oner[:, 0:128], e_row[:],
                                     start=True, stop=True)
                    e_sb = work.tile([128, NLOC], F32, name="e_sb_0",
                                     tag="e_sb", bufs=2)
                    nc.vector.tensor_copy(e_sb[:], e_ps[:])

                # ---- affine part: va[dh] = Aaff_t @ z_loc + b_t ---------
                va_sb = []
                aaff3 = aaff_sb[:].rearrange("k (r d) -> k r d", r=2)
                zb3a = zb[:].rearrange("k (r i) -> k r i", r=2)
                for dh in (0, 1):
                    va = psum.tile([128, NLOC], F32, name=f"va_{t}_{dh}",
                                   tag="aux", bufs=2)
                    nc.tensor.matmul(va[:],
                                     aaff3[:, :, dh * 128:(dh + 1) * 128],
                                     zb3a[:], start=True, stop=False,
                                     perf_mode=DR)
                    nc.tensor.matmul(va[:],
                                     brow_t[:, dh * 128:(dh + 1) * 128],
                                     oner[:], start=False, stop=True)
                    # va holds SAF*(Aaff@z + b): descale, then add z master
                    vs0 = work.tile([128, NLOC], F32, name=f"vs0_{t}_{dh}",
                                    tag=f"vs0{dh}", bufs=2)
                    nc.vector.tensor_scalar_mul(vs0[:], va[:], 1.0 / SAF)
                    vsb = work.tile([128, NLOC], F32, name=f"vasb_{t}_{dh}",
                                    tag=f"vasb{dh}", bufs=2)
                    nc.vector.tensor_add(vsb[:], vs0[:], zt[dh][:])
                    va_sb.append(vsb)

                # ---- A' = A * exp(-c*sq_j): per-partition rescale -------
                a_sc = astream.tile([128, NJB * D], F8, name=f"asc_{t}",
                                    tag="asc")
                for jb in range(NJB):
                    nc.vector.tensor_scalar_mul(
                        a_sc[:, jb * D:(jb + 1) * D],
                        a_sb[:, jb * D:(jb + 1) * D],
                        e_cols[t % 2][:, jb:jb + 1])

                # ---- main loop over block PAIRS (DoubleRow, fp8) --------
                NPAIR = NJB // 2
                ztf3 = ztf[:].rearrange("k (r j) -> k r j", r=2)
                zb3 = zb[:].rearrange("k (r i) -> k r i", r=2)
                vr = [psum.tile([128, NLOC], F32, name=f"vr_{t}_{dh}",
                                tag=f"vr{dh}", bufs=1) for dh in (0, 1)]
                k_pairs = [None] * NPAIR
                for qq in range(NPAIR + PIPE_LAG):
                    if qq < NPAIR:
                        k_p = kpool.tile([128, 2 * NLOC], F8,
                                         name=f"k_{t}_{qq}", tag="k")
                        s_ps = psum.tile([128, 2 * NLOC], F32,
                                         name=f"s_{t}_{qq}",
                                         tag="s", bufs=2)
                        for h in (0, 1):
                            jb = 2 * qq + h
                            nc.tensor.matmul(
                                s_ps[:, h * NLOC:(h + 1) * NLOC],
                                ztf3[:, :, jb * 128:(jb + 1) * 128],
                                zb3[:], start=True, stop=True,
                                perf_mode=DR)
                        # G = exp(2cS); exp(-c sq_j) is pre-folded into A,
                        # exp(-c sq_i) post-multiplies vr
                        nc.scalar.activation(k_p[:], s_ps[:],
                                             EXP, scale=2.0 * CEXP)
                        k_pairs[qq] = k_p
                    if qq >= PIPE_LAG:
                        q = qq - PIPE_LAG
                        k_p = k_pairs[q]
                        a3 = a_sc[:, 2 * q * D:2 * (q + 1) * D].rearrange(
                            "k (r d) -> k r d", r=2)
                        k3 = k_p[:].rearrange("k (r i) -> k r i", r=2)
                        for dh in (0, 1):
                            nc.tensor.matmul(
                                vr[dh][:],
                                a3[:, :, dh * 128:(dh + 1) * 128],
                                k3[:],
                                start=(q == 0),
                                stop=(q == NPAIR - 1 and dh == 1),
                                perf_mode=DR)

                # keep TensorE busy across the DVE update window (the
                # clock-gate re-throttles on any PE idle) — idempotent,
                # overwritten by the real fillers in the tail
                if not last:
                    e_ps_next = psum.tile([128, NLOC], F32,
                                          name=f"e_ps_{t + 1}",
                                          tag="aux", bufs=2)
                    for _f in range(PRE_FILLERS):
                        nc.tensor.matmul(e_ps_next[:], oner[:, 0:128],
                                         e_row[:], start=True, stop=True)

                # ---- update: z <- z + va + vr * E -----------------------
                zt_new = [state.tile([128, NLOC], F32, name=f"ztn_{t}_{ch}",
                                     tag=f"zt{ch}") for ch in (0, 1)]
                for dh in (0, 1):
                    t1 = work.tile([128, NLOC], F32, name=f"t1_{t}_{dh}",
                                   tag="t1", bufs=2)
                    nc.vector.tensor_mul(t1[:], vr[dh][:], e_sb[:])
                    nc.vector.tensor_add(zt_new[dh][:], t1[:], va_sb[dh][:])
                zt = zt_new

                if last:
                    for ch in (0, 1):
                        nc.sync.dma_start(
                            out_zt[ch * 128:(ch + 1) * 128, :], zt[ch][:])
                    break

                # ---- post-update tail: bf16 copy, sq, payload, AG -------
                zb_new = state.tile([128, 2 * NLOC], F8, name=f"zbn_{t}",
                                    tag="zb")
                z2 = [work.tile([128, NLOC], BF16, name=f"z2_{t}_{ch}",
                                tag=f"z2{ch}", bufs=2) for ch in (0, 1)]
                for ch in (0, 1):
                    nc.vector.tensor_copy(
                        zb_new[:, ch * NLOC:(ch + 1) * NLOC], zt[ch][:])
                    nc.scalar.activation(z2[ch][:], zt[ch][:], SQUARE)
                zb = zb_new

                # sq in column layout [128, 4] (for payload -> bias)
                sqc_ps = psum.tile([128, NJB_LOC], F32, name=f"sqc_{t}",
                                   tag="aux", bufs=2)
                for ib in range(NJB_LOC):
                    for ch in (0, 1):
                        nc.tensor.matmul(sqc_ps[:, ib:ib + 1],
                                         z2[ch][:, ib * 128:(ib + 1) * 128],
                                         onec[:],
                                         start=(ch == 0), stop=(ch == 1))
                sqc_b = work.tile([128, NJB_LOC], BF16, name=f"sqcb_{t}",
                                  tag="sqcb", bufs=2)
                nc.vector.tensor_copy(sqc_b[:], sqc_ps[:])

                # sq in row layout [1, 512] -> E row for next step
                sqr_ps = psum.tile([1, NLOC], F32, name=f"sqr_{t}",
                                   tag="aux", bufs=2)
                for ch in (0, 1):
                    nc.tensor.matmul(sqr_ps[:], onec[:], z2[ch][:],
                                     start=(ch == 0), stop=(ch == 1))
                e_row_new = state.tile([1, NLOC], BF16, name=f"er_{t}",
                                       tag="e_row")
                nc.scalar.activation(e_row_new[:], sqr_ps[:], EXP, scale=-CEXP,
                                     bias=lsa[:])
                e_row = e_row_new

                # ---- pack payload + AllGather ---------------------------
                cc_in = dram.tile([PAY_R, NLOC], F8, name=f"cci_{t}",
                                  tag="cci")
                cc_out = dram.tile([N_CORES * PAY_R, NLOC], F8,
                                   name=f"cco_{t}", tag="cco",
                                   addr_space="Shared")
                for ch in (0, 1):
                    nc.sync.dma_start(cc_in[ch * 128:(ch + 1) * 128, :],
                                      zb[:, ch * NLOC:(ch + 1) * NLOC])
                nc.sync.dma_start(
                    cc_in[D:D + 2, :].rearrange("o (q b) -> (o q) b", b=8),
                    sqc_b[:].bitcast(F8))
                nc.gpsimd.collective_compute(
                    "AllGather", mybir.AluOpType.bypass,
                    replica_groups=[list(range(N_CORES))],
                    ins=[cc_in[:].opt()], outs=[cc_out[:].opt()],
                )

                # ---- unpack gathered Z^T, sq (one DMA each) -------------
                cco3 = cc_out[:].rearrange("(r q) i -> r q i", r=N_CORES)
                HR = N_CORES // 2
                for half in (0, 1):
                    for ch in (0, 1):
                        nc.sync.dma_start(
                            ztf[:, ch * N + half * HR * NLOC:
                                ch * N + (half + 1) * HR * NLOC]
                            .rearrange("p (r i) -> p r i", r=HR),
                            cco3[half * HR:(half + 1) * HR,
                                 ch * 128:(ch + 1) * 128, :]
                            .rearrange("r p i -> p r i"))
                nc.sync.dma_start(
                    sqc_all[:].bitcast(F8).rearrange("p (r b) -> p r b",
                                                     r=N_CORES),
                    cco3[:, D:D + 2, :]
                    .rearrange("r o (q b) -> (o q) r b", b=8))
                # bias = -c * sq  (fp32, per-partition columns)
                nc.vector.tensor_scalar_mul(bias_col[:], sqc_all[:], -CEXP)
                nc.scalar.activation(e_cols[t % 2][:], bias_col[:], EXP)

                # next step's E broadcast, repeated FILLER_MMS times: the
                # matmul is idempotent (start=True each time), and the
                # repetitions keep TensorE's HAM clock-gate warm while the
                # AllGather + unpack are in flight.
                for _f in range(FILLER_MMS):
                    nc.tensor.matmul(e_ps_next[:], oner[:, 0:128], e_row[:],
                                     start=True, stop=True)
                e_sb = work.tile([128, NLOC], F32, name=f"e_sb_{t + 1}",
                                 tag="e_sb", bufs=2)
                nc.vector.tensor_copy(e_sb[:], e_ps_next[:])

    nc.compile()
    return nc


def _prepare_in_maps(X, A, A_aff, b_aff):
    XT = np.ascontiguousarray(X.T.astype(np.float32))          # [D, N]
    sq0 = (X.astype(np.float32) ** 2).sum(axis=1)              # [N]
    ztb_full0 = XT.astype(F8NP)
    bias_col0 = np.ascontiguousarray(
        (-CEXP * sq0).reshape(NJB, 128).T.astype(np.float32))  # [128, 32]
    a_b = (DT * SA * A.astype(np.float32)).astype(F8NP)        # [T, N, D]
    aaff_b = np.ascontiguousarray(
        (DT * SAF * A_aff.astype(np.float32)).transpose(0, 2, 1)).astype(F8NP)
    b_b = (DT * SAF * b_aff.astype(np.float32)).reshape(T, 1, D).astype(BF16NP)
    ones_col = np.ones((128, 1), dtype=BF16NP)
    ones_row = np.ones((1, NLOC), dtype=BF16NP)

    in_maps = []
    for c in range(N_CORES):
        cols = slice(c * NLOC, (c + 1) * NLOC)
        zt_local0 = np.ascontiguousarray(XT[:, cols])
        in_maps.append({
            "zt_local0": zt_local0,
            "zb_local0": zt_local0.astype(F8NP),
            "ztb_full0": ztb_full0,
            "bias_col0": bias_col0,
            "e_row0": (np.exp(-CEXP * sq0[cols]) / SA)[None, :].astype(BF16NP),
            "a_b": a_b,
            "aaff_b": aaff_b,
            "b_b": b_b,
            "ones_col": ones_col,
            "ones_row": ones_row,
            "log_inv_sa": np.array([[np.log(1.0 / SA)]], dtype=np.float32),
        })
    return in_maps


def _get_nc():
    if "nc" not in _CACHED:
        _CACHED["nc"] = _build()
    return _CACHED["nc"]


def kernel(X, A, A_aff, b_aff):
    X = np.asarray(X)
    A = np.asarray(A)
    A_aff = np.asarray(A_aff)
    b_aff = np.asarray(b_aff)
    nc = _get_nc()
    in_maps = _prepare_in_maps(X, A, A_aff, b_aff)
    res = bass_utils.run_bass_kernel_spmd(
        nc, in_maps, core_ids=list(range(N_CORES)))
    out = np.empty((N, D), dtype=np.float32)
    for c in range(N_CORES):
        out[c * NLOC:(c + 1) * NLOC, :] = res.results[c]["out_zt"].T
    return out

